# revision 40
# baseline (speedup 1.0000x reference)
"""AttnBlock (GroupNorm + single-head self-attention + residual) on 8 TRN2 cores.

Sharding: data-parallel over (batch b, query-half h) -> 8 shards. Each core
receives the full [C, N] image of its batch (columns rolled so that its own
query half always occupies columns 0:NQ), computes GroupNorm stats + K/V over
the whole image, Q over its half, and a flash-style attention in which scores
are produced directly transposed (S^T = K^T.T @ Q^T tiles) so softmax
normalization is done via a ones-vector matmul and no PE transposes of P are
needed.

All large matmuls (projections, S^T, PV, denominator, out-proj) run as fp8e4
DoubleRow matmuls: lhsT [128, 2, M] / rhs [128, 2, N] contract 256 deep in a
single instruction at ~2x bf16 FLOP rate. Weights are scaled x8 before the
fp8 cast (their entries are ~N(0, 1/16) and would hit e4m3 subnormals); the
scale is removed in the PSUM->SBUF cast. The softmax exp is shifted by -2
(exp(s/16 - 2)) so P fits e4m3's +-240 range; numerator and denominator share
the shift so the ratio is unchanged. exp runs on ACT in 2-key-tile batches
([128,1024] over a 2-bank PSUM tile) to amortize the per-call overhead, and
the denominator is a DoubleRow ones-matmul on the PE (accumulated in a
[1,512] PSUM bank), keeping the DVE free for casts and the epilogue.

Head/tail schedule (the attention loop itself runs at the PE's hardware pace,
~227ns per 512-free DoubleRow matmul):
- GroupNorm stats split DVE (13 bn_stats units) + ACT (3 units as Square /
  Identity passes with free-axis accum_out); a small leading x8 DMA chunk
  starts the stream early. All table sets contain identity/copy/square, so
  the sqrt set serves stats+dance and exp loads once, pinned into the
  projection phase by a fake bo_eff dependency on the dummy exp (the tile
  scheduler otherwise hoists it and the table sets ping-pong).
- The first K projection pair is issued right after wkt is scaled; the bias
  matvec chain rides the PE behind it. wo8 is pinned behind the dance via a
  zero-column bias dep (unpinned, its wstg wait stalls the ACT stats).
- A wstg-dependent warm-matmul block keeps HAM at full clock through the
  dance window; the final block's output DMA is halved across both HWDGE
  queues.
Hard-won constraints: Pool-engine (gpsimd) ucode tensor ops force library
swaps (UNLOAD/LOAD + multi-us drains) that stall the whole core — gpsimd may
only run partition_broadcast + DMA issues. Each DMA issue on the scalar
queue costs ~0.7us of ACT sequencer time. Per-queue DMA is ~90B/ns with
>=1KB contiguous runs, roughly half that for channel-split chunks.
"""

import os
import sys

import numpy as np

for _p in ("/opt/trn_rl_repo", "/root/.axon_site/_ro/trn_rl_repo"):
    if os.path.isdir(_p) and _p not in sys.path:
        sys.path.insert(0, _p)

import concourse.bass as bass  # noqa: E402
import concourse.tile as tile  # noqa: E402
from concourse import bacc, mybir  # noqa: E402
from concourse.masks import make_identity  # noqa: E402

# The agent image's antenv lacks axon_hooks; if BASS_TRACE is set in the
# environment, run_bass_kernel_spmd would crash importing it. Provide a stub
# (profiling degrades gracefully to "hook isn't registered").
try:
    import antenv.axon_hooks  # noqa: F401
except ImportError:
    import types as _types

    _m = _types.ModuleType("antenv.axon_hooks")
    _h = [None]
    _m.set_axon_ntff_profile_hook = lambda h: _h.__setitem__(0, h)
    _m.get_axon_ntff_profile_hook = lambda: _h[0]
    sys.modules["antenv.axon_hooks"] = _m

B, C, H, W = 4, 256, 64, 64
N = H * W  # 4096 pixels
NQ = N // 2  # 2048 queries per core
G = 32  # groups
CPG = C // G  # 8 channels per group
EPS = 1e-5
NCORES = 8
SCALE = float(C) ** -0.5  # 0.0625
ESHIFT = -3.0  # exp(s*SCALE + ESHIFT): data max logit 7.95 -> P <= ~141 < 240
WS = 8.0  # weight fp8 pre-scale (entries ~N(0,1/16) need lifting)

F32 = mybir.dt.float32
BF16 = mybir.dt.bfloat16
FP8 = mybir.dt.float8e4

QB = 512  # query block (free dim of S^T / PV matmuls)
NQB = NQ // QB  # 4 query blocks
NKT = N // 128  # 32 key tiles
NKP = NKT // 2  # 16 key-tile pairs (DoubleRow granularity)
NNB = N // QB  # 8 pixel blocks for K/V projections
P = 128

DEBUG = bool(int(os.environ.get("KDEBUG", "0")))

Act = mybir.ActivationFunctionType
Alu = mybir.AluOpType
Axis = mybir.AxisListType
DR = mybir.MatmulPerfMode.DoubleRow

_NC = None
LAST_RESULTS = None


def _body(tc, d):
    nc = tc.nc
    x_d = d["x"]
    out_d = d["out"]

    const = tc.alloc_tile_pool(name="const", bufs=1)
    stage = tc.alloc_tile_pool(name="stage", bufs=2)
    small = tc.alloc_tile_pool(name="small", bufs=1)
    pblk = tc.alloc_tile_pool(name="pblk", bufs=2)
    work = tc.alloc_tile_pool(name="work", bufs=2)
    # PSUM budget (8 banks): S-pair tiles 2x[P,1024] = 4, aps 2x[P,512] = 2,
    # dps [1,512] = 1, po [P,512] = 1.
    ps = tc.alloc_tile_pool(name="ps", bufs=2, space="PSUM")
    ps_acc = tc.alloc_tile_pool(name="ps_acc", bufs=2, space="PSUM")
    ps_d = tc.alloc_tile_pool(name="ps_d", bufs=1, space="PSUM")
    ps_o = tc.alloc_tile_pool(name="ps_o", bufs=1, space="PSUM")

    # ---- x in SBUF first: bf16 [128, 2(ch), 4096] (residual + stats) and
    # fp8 [128, 2, 4096] (matmul operand), both cast on host; chunked so
    # bn_stats overlaps the transfer ----
    x_sb = const.tile([P, 2, N], BF16)
    x8 = const.tile([P, 2, N], FP8)
    x_src = x_d.ap().rearrange("(h p) n -> p h n", p=P)
    x8_src = d["x8"].ap().rearrange("(h p) n -> p h n", p=P)

    # PE warm-up: keep the HAM activity monitor busy during the DMA/stats
    # window so projections and attention run at full clock from the start.
    # Memsets go FIRST so the warm matmuls aren't queued behind the bn_stats
    # chain in DVE program order (PE would idle for the whole DMA phase).
    wu_w = const.tile([P, P], BF16)
    nc.vector.memset(wu_w, 0.0)
    wu_x = const.tile([P, 2 * P], BF16)
    nc.vector.memset(wu_x, 0.0)
    wu_ps = ps.tile([P, QB], F32, name="wu_ps", tag="mm")

    def warm(n):
        for _ in range(n):
            nc.tensor.matmul(
                wu_ps[:, 0:2 * P], lhsT=wu_w, rhs=wu_x, start=True, stop=True
            )

    warm(2)

    # ---- head DMA: x8 (1MB, feeds stats AND all matmuls) goes FIRST in
    # 1024-col per-channel chunks: ch1 leads on both queues so the ACT
    # stats units and the DVE's first units have data earliest. ACT's own
    # HWDGE queue carries NO early DMA (each issue costs ~0.7us of ACT
    # sequencer time and starved the act-table loads in an earlier rev):
    # only the weight pack + one x_sb chunk ride scalar, the rest goes
    # sync + gpsimd (SWDGE).
    # Both-channel column chunks (1024B+ contiguous per partition transfers
    # ~90B/ns/queue; channel-split halves that). A small 512-col first
    # chunk lands ~1.5us earlier so the stats stream starts sooner.
    x8_chunks = [
        (nc.sync, 0, 1), (nc.scalar, 1, 2), (nc.gpsimd, 4, 6),
        (nc.sync, 2, 4), (nc.gpsimd, 6, 8),
    ]
    for q, u0, u1 in x8_chunks:
        sl = (slice(None), slice(None), slice(u0 * QB, u1 * QB))
        q.dma_start(out=x8[sl], in_=x8_src[sl])
    # all four weight matrices (bf16) in one DMA on the scalar queue (its
    # only early issue; needed at ~17us for the w_s scaling)
    wstg = stage.tile([P, 4, 2, C], BF16, name="wstg", tag="wstg")
    nc.scalar.dma_start(
        out=wstg, in_=d["wpack"].ap().rearrange("(w h p) co -> p w h co", p=P, h=2)
    )
    m1_sb = const.tile([P, G // 2], F32)
    nc.sync.dma_start(out=m1_sb, in_=d["m1"][:, :])
    m2_sb = const.tile([G // 2, P], F32)
    nc.sync.dma_start(out=m2_sb, in_=d["m2"][:, :])
    # packed biases/affine: rows (bq,bk,bv,bo,gamma,beta) -> per-channel cols
    bcols = const.tile([P, 2, 6], F32)
    nc.sync.dma_start(
        out=bcols, in_=d["bpack"].ap().rearrange("(h p) s -> p h s", p=P)
    )
    xsbq = [nc.sync, nc.sync, nc.gpsimd, nc.gpsimd]
    for c in range(4):
        sl = (slice(None), slice(None), slice(c * 2 * QB, (c + 1) * 2 * QB))
        xsbq[c].dma_start(out=x_sb[sl], in_=x_src[sl])

    # GroupNorm stats from the fp8 x (self-consistent: the projections
    # consume the same fp8 values; adds ~0.1% to the error budget). Split
    # across DVE (bn_stats: ch0 + ch1 units 0,1,2,3,7) and ACT (Square +
    # Identity passes with free-axis accumulate: ch1 units 4,5,6). Every
    # ACT table set contains identity+copy+square, so the sqrt set
    # (preloaded by the dummy sqrt below) serves the stats AND the dance;
    # the only other table load is the exp set, pinned into the projection
    # phase by the dummy exp's fake bo_eff dependency.
    eps16 = const.tile([G // 2, 1], F32)
    nc.vector.memset(eps16, EPS)
    one11 = const.tile([1, 1], F32)
    nc.vector.memset(one11, 1.0)
    warm11 = small.tile([1, 1], F32)
    nc.scalar.activation(warm11, one11, Act.Sqrt, scale=1.0)

    bn_st0 = small.tile([P, NNB, 6], F32, name="bn_st0")
    bn_st1b = small.tile([P, 5, 6], F32, name="bn_st1b")
    ACT_UNITS = (4, 5, 6)
    sq5 = small.tile([P, len(ACT_UNITS)], F32, name="sq5")
    sm5 = small.tile([P, len(ACT_UNITS)], F32, name="sm5")
    scr_bf = stage.tile([P, QB], BF16, name="scr_bf", tag="scr")
    for i, u in enumerate(ACT_UNITS):
        cols = slice(u * QB, (u + 1) * QB)
        nc.scalar.activation(
            scr_bf, x8[:, 1, cols], Act.Square, accum_out=sq5[:, i:i + 1]
        )
        nc.scalar.activation(
            scr_bf, x8[:, 1, cols], Act.Identity, accum_out=sm5[:, i:i + 1]
        )
    # DVE bn_stats in data-arrival order: small first chunk (u0), u1,
    # then gpsimd chunk (ch0 u4,u5), sync chunk (u2,u3), gpsimd tail.
    dve_units = [(1, 0, 0), (0, 0, None), (1, 1, 1), (0, 1, None),
                 (0, 4, None), (0, 5, None),
                 (1, 2, 2), (1, 3, 3), (0, 2, None), (0, 3, None),
                 (1, 7, 4), (0, 6, None), (0, 7, None)]
    for i, (ch, u, b1) in enumerate(dve_units):
        dst = bn_st0[:, u, :] if ch == 0 else bn_st1b[:, b1, :]
        nc.vector.bn_stats(out=dst, in_=x8[:, ch, u * QB:(u + 1) * QB])
        if i % 2 == 1:
            warm(6)

    # ---- constants ----
    # padded to 16B so the DoubleRow k-tile stride meets walrus' step%16==0.
    # Value 1.0 balances the at_sb 1/8 down-scale and the x8 wo lift:
    # po*den_r = (8/8)*wo@at_un / den = wo@at_un/den.
    ones8_pad = const.tile([P, 2, 16], FP8)
    nc.vector.memset(ones8_pad, 1.0)
    ones8 = ones8_pad[:, :, 0:1]
    esh_col = const.tile([P, 1], F32)
    nc.vector.memset(esh_col, ESHIFT)

    # ---- GroupNorm dance: per-channel (mean, E[x^2]) -> group reduce via
    # mask matmul -> rstd -> broadcast back via mask matmul -> a/b columns.
    # ch0 comes straight from bn_aggr; ch1 blends the 5 DVE units (2560
    # cols, weight 0.625) with the ACT accumulator sums (1536 cols).
    mvex = small.tile([P, 2, 2], F32, name="mvex")  # (mean, ex2) per ch-half
    mv0 = small.tile([P, 2], F32, name="mv0")
    nc.vector.bn_aggr(out=mv0, in_=bn_st0)
    mv1b = small.tile([P, 2], F32, name="mv1b")
    nc.vector.bn_aggr(out=mv1b, in_=bn_st1b)
    scol = small.tile([P, 2], F32, name="scol")
    nc.vector.tensor_reduce(
        out=scol[:, 0:1], in_=sm5, axis=Axis.X, op=Alu.add
    )
    nc.vector.tensor_reduce(
        out=scol[:, 1:2], in_=sq5, axis=Axis.X, op=Alu.add
    )
    tcol = small.tile([P, 6], F32, name="tcol")
    # ch0: mean, ex2 = var + mean^2
    nc.vector.tensor_copy(out=mvex[:, 0, 0:1], in_=mv0[:, 0:1])
    nc.vector.tensor_mul(tcol[:, 0:1], mv0[:, 0:1], mv0[:, 0:1])
    nc.vector.tensor_add(mvex[:, 0, 1:2], tcol[:, 0:1], mv0[:, 1:2])
    # ch1: mean = FB*mean_b + sum_id/4096
    FB = 2560.0 / 4096.0
    nc.vector.tensor_scalar_mul(tcol[:, 1:2], scol[:, 0:1], 1.0 / 4096.0)
    nc.vector.scalar_tensor_tensor(
        out=mvex[:, 1, 0:1], in0=mv1b[:, 0:1], scalar=FB, in1=tcol[:, 1:2],
        op0=Alu.mult, op1=Alu.add,
    )
    # ch1: ex2 = FB*(var_b + mean_b^2) + sum_sq/4096
    nc.vector.tensor_mul(tcol[:, 2:3], mv1b[:, 0:1], mv1b[:, 0:1])
    nc.vector.tensor_add(tcol[:, 3:4], mv1b[:, 1:2], tcol[:, 2:3])
    nc.vector.tensor_scalar_mul(tcol[:, 4:5], scol[:, 1:2], 1.0 / 4096.0)
    nc.vector.scalar_tensor_tensor(
        out=mvex[:, 1, 1:2], in0=tcol[:, 3:4], scalar=FB, in1=tcol[:, 4:5],
        op0=Alu.mult, op1=Alu.add,
    )
    # group sums over partitions: [16 groups, (mean0, ex20, mean1, ex21)]
    gsum = ps_o.tile([G // 2, 4], F32, name="gsum", tag="po")
    nc.tensor.matmul(
        gsum, lhsT=m1_sb, rhs=mvex.rearrange("p c k -> p (c k)"),
        start=True, stop=True,
    )
    warm(8)
    vals = small.tile([G // 2, 4], F32, name="vals")  # (rstd0, rstd1, m0, m1)
    gtmp = small.tile([G // 2, 4], F32, name="gtmp")
    gview = gsum.rearrange("g (c k) -> g c k", k=2)
    nc.vector.tensor_scalar_mul(
        vals.rearrange("g (c k) -> g c k", k=2)[:, 1, :], gview[:, :, 0],
        1.0 / CPG,
    )
    nc.vector.tensor_scalar_mul(gtmp[:, 0:2], gview[:, :, 1], 1.0 / CPG)
    nc.vector.tensor_mul(gtmp[:, 2:4], vals[:, 2:4], vals[:, 2:4])
    nc.vector.tensor_sub(gtmp[:, 0:2], gtmp[:, 0:2], gtmp[:, 2:4])
    # rstd = 1/sqrt(var + eps) (sqrt table preloaded; recip on DVE)
    nc.scalar.activation(gtmp[:, 2:4], gtmp[:, 0:2], Act.Sqrt, bias=eps16)
    nc.vector.reciprocal(vals[:, 0:2], gtmp[:, 2:4])
    # broadcast back to channels: [128, (rstd0, rstd1, m0, m1)]
    bc = ps_acc.tile([P, 4], F32, name="bc", tag="acc")
    nc.tensor.matmul(bc, lhsT=m2_sb, rhs=vals, start=True, stop=True)
    warm(8)
    # a = gamma * rstd; b = beta - mean * a; a8 = 8a (per channel cols)
    cols8 = small.tile([P, 8], F32, name="cols8")
    a_cols = [cols8[:, 0:1], cols8[:, 1:2]]
    b_cols = [cols8[:, 2:3], cols8[:, 3:4]]
    a8_cols = [cols8[:, 4:5], cols8[:, 5:6]]
    for ch in range(2):
        nc.vector.tensor_mul(a_cols[ch], bcols[:, ch, 4:5], bc[:, ch:ch + 1])
        nc.vector.tensor_mul(cols8[:, 6 + ch:7 + ch], bc[:, 2 + ch:3 + ch],
                             a_cols[ch])
        nc.vector.tensor_sub(b_cols[ch], bcols[:, ch, 5:6],
                             cols8[:, 6 + ch:7 + ch])
        nc.vector.tensor_scalar_mul(a8_cols[ch], a_cols[ch], WS)
    warm(8)
    # a zero column that data-depends on the dance: used to PIN late ACT
    # ops (wo8) behind the stats/dance — without a dep the scheduler
    # hoists them into the ACT stats stream where their wstg wait stalls it.
    zcol = small.tile([P, 1], F32, name="zcol")
    nc.vector.tensor_scalar_mul(zcol, a_cols[0], 0.0)
    # HAM filler through the dance: the dep-free warm matmuls all get
    # scheduled during the stats phase, so the PE goes quiet for the
    # ~17-22us dance window and HAM halves the clock right as the
    # projections start. These reads of the (just-landed) weight pack are
    # naturally pinned to that window.
    for i in range(28):
        nc.tensor.matmul(
            wu_ps[:, 0:C], lhsT=wu_w, rhs=wstg[:, i % 4, i % 2, :],
            start=True, stop=True,
        )

    # scale wq/wk/wv rows by 8*a (per input channel) straight from the f32
    # staging into fp8 tiles. The x8 lift keeps the fp8 entries
    # (~N(0, a/16)) out of e4m3 subnormal range; the PSUM->SBUF casts
    # divide it back out.
    w_s = {}
    for wi, wname in ((0, "wqt"), (1, "wkt"), (2, "wvt")):
        w_s[wname] = const.tile([P, 2, C], FP8, name=f"{wname}_s")
    # wk first: it gates the longest projection stream (all on DVE — the
    # Pool engine's ucode tensor ops force library swaps that stall the
    # whole core).
    for wname, wi in (("wkt", 1), ("wqt", 0), ("wvt", 2)):
        for ci in range(2):
            nc.vector.tensor_scalar_mul(
                w_s[wname][:, ci, :], wstg[:, wi, ci, :], a8_cols[ci]
            )
    wo8 = const.tile([P, 2, C], FP8)

    # ---- projections (all DoubleRow fp8, contraction over full C=256) ----
    # K^T [C, N] fp8: psum[co,nb] = sum_ci wkt8[ci,co].T @ x8[ci, nb] (x8)
    # and q,k = psum/8 + bias. Casts alternate ACT / DVE (ACT is idle until
    # the attention loop's exp stream starts). The first K matmul pair is
    # issued the moment wkt is scaled; the tiny bias matvecs then ride the
    # PE behind it so their serial chain overlaps the projection stream.
    k_sb = const.tile([P, 2, N], FP8)
    q_sb = const.tile([P, 2, NQ], FP8)
    IWS = 1.0 / WS
    proj_ps = [(ps, "mm"), (ps_acc, "acc"), (ps, "mm"), (ps_o, "po"),
               (ps_acc, "acc")]
    _pi = [0]

    def proj_tile():
        pool, tag = proj_ps[_pi[0] % len(proj_ps)]
        _pi[0] += 1
        return pool.tile([P, QB], F32, name="pp", tag=tag)

    _ci = [0]

    def proj_cast(out, in_, bias):
        # only ACT/DVE can read PSUM; the 5-bank psum ring above keeps the
        # MM stream ~2.5 tiles ahead so the 2-engine cast pace (~325ns/tile)
        # doesn't stall the PE.
        i = _ci[0] % 2
        _ci[0] += 1
        if i == 0:
            nc.scalar.activation(out, in_, Act.Identity, bias=bias, scale=IWS)
        else:
            nc.vector.tensor_scalar(
                out=out, in0=in_, scalar1=IWS, scalar2=bias, op0=Alu.mult,
                op1=Alu.add,
            )

    # first K pair (block 0) — only these two proj psums may be in flight
    # before the matvec chain (deeper would deadlock through be_*'s ring
    # reuse of the same PSUM tags).
    pk0 = [proj_tile() for _ in range(2)]
    for co in range(2):
        nc.tensor.matmul(
            pk0[co], lhsT=w_s["wkt"][:, :, co * P:(co + 1) * P],
            rhs=x8[:, :, 0:QB], start=True, stop=True, perf_mode=DR,
        )

    # projection bias columns: be = W b + bias (bf16 matvecs off the staged
    # weights; psum ring across three pools so the chain pipelines)
    b_bf = small.tile([P, 2], BF16, name="b_bf")
    for ch in range(2):
        nc.vector.tensor_copy(out=b_bf[:, ch:ch + 1], in_=b_cols[ch])
    _mvi = [0]
    mv_pools = [(ps_o, "po"), (ps_acc, "acc"), (ps, "mm")]

    def matvec_bias(wi, rhs_cols, bias_idx, out_name, out_dt=F32):
        outs = []
        for co in range(2):
            pool, tag = mv_pools[_mvi[0] % 3]
            _mvi[0] += 1
            pe = pool.tile([P, 1], F32, name="pe_mv", tag=tag)
            for ci in range(2):
                nc.tensor.matmul(
                    pe, lhsT=wstg[:, wi, ci, co * P:(co + 1) * P],
                    rhs=rhs_cols[ci], start=(ci == 0), stop=(ci == 1),
                )
            t = small.tile([P, 1], out_dt, name=f"{out_name}_{co}")
            nc.scalar.activation(
                t, pe, Act.Identity, bias=bcols[:, co, bias_idx:bias_idx + 1],
                scale=1.0,
            )
            outs.append(t)
        return outs

    bcol_list = [b_bf[:, 0:1], b_bf[:, 1:2]]
    be_q = matvec_bias(0, bcol_list, 0, "be_q")
    be_k = matvec_bias(1, bcol_list, 1, "be_k")
    vbv = matvec_bias(2, bcol_list, 2, "vbv", out_dt=BF16)
    bo_eff = matvec_bias(3, vbv, 3, "bo_eff")
    # dummy exp: pulls the exp table load into the projection phase. The
    # scale AP is a fake data dependency on bo_eff — without it the tile
    # scheduler hoists this exp into the stats phase and the sqrt/exp
    # table sets ping-pong (4 extra 1.5us table loads).
    nc.scalar.activation(warm11, one11, Act.Exp, scale=bo_eff[0][0:1, 0:1])
    for co in range(2):
        proj_cast(k_sb[:, co, 0:QB], pk0[co], be_k[co])
    # wo8 = 8*wo in fp8 (no GroupNorm folding on the out-proj); the zcol
    # bias pins these behind the dance (first consumer is qb0's epilogue,
    # ~20us later).
    for ch in range(2):
        nc.scalar.activation(wo8[:, ch, :], wstg[:, 3, ch, :], Act.Identity,
                             bias=zcol, scale=WS)

    for nb in range(NNB):
        for co in range(2):
            if nb < NQB:
                pq = proj_tile()
                nc.tensor.matmul(
                    pq, lhsT=w_s["wqt"][:, :, co * P:(co + 1) * P],
                    rhs=x8[:, :, nb * QB:(nb + 1) * QB],
                    start=True, stop=True, perf_mode=DR,
                )
                proj_cast(q_sb[:, co, nb * QB:(nb + 1) * QB], pq, be_q[co])
            if nb >= 1:
                pk = proj_tile()
                nc.tensor.matmul(
                    pk, lhsT=w_s["wkt"][:, :, co * P:(co + 1) * P],
                    rhs=x8[:, :, nb * QB:(nb + 1) * QB],
                    start=True, stop=True, perf_mode=DR,
                )
                proj_cast(k_sb[:, co, nb * QB:(nb + 1) * QB], pk, be_k[co])

    # V [N, C] fp8 (bias folded into bo_eff): psum[nt] = x8_tile.T @ wvt8
    v_sb = const.tile([P, NKT, C], FP8)
    v_flat = v_sb.rearrange("p k c -> p (k c)")
    zero_col = const.tile([P, 1], F32)
    nc.vector.memset(zero_col, 0.0)
    for nt in range(0, NKT, 2):
        pv = proj_tile()
        for n2 in range(2):
            nc.tensor.matmul(
                pv[:, n2 * C:(n2 + 1) * C],
                lhsT=x8[:, :, (nt + n2) * P:(nt + n2 + 1) * P],
                rhs=w_s["wvt"][:, :, :],
                start=True, stop=True, perf_mode=DR,
            )
        proj_cast(v_flat[:, nt * C:(nt + 2) * C], pv, zero_col)

    # ---- attention, per query block; DoubleRow over key-tile pairs with a
    # batched exp (one ACT call per pair reading a 2-bank PSUM tile) and the
    # denominator as a DoubleRow ones-matmul on the PE. The softmax division
    # is commuted through the out-projection: out = (wo8 @ PV) * (1/(8*den))
    # + bo_eff + x, deferred one qb so nothing waits on the reciprocal.
    def epilogue_a(qb, dps, aps, at_sb, p_sb_dbg=None):
        # casts first: they release the PV accumulator banks immediately.
        # 1/8 keeps the heavy-tailed PV numerator inside fp8's +-240.
        nc.vector.tensor_scalar_mul(at_sb[:, 0, :], aps[0], 1.0 / 8.0)
        nc.vector.tensor_scalar_mul(at_sb[:, 1, :], aps[1], 1.0 / 8.0)
        den_r = work.tile([1, QB], F32, name="den_r", tag="den_r")
        nc.vector.reciprocal_approx_fast(out=den_r, in_=dps)
        den_b = work.tile([P, QB], F32, name="den_b", tag="den_b", bufs=2)
        nc.gpsimd.partition_broadcast(den_b, den_r)
        if DEBUG:
            nc.sync.dma_start(
                out=d["dbg_denr"][:, qb * QB:(qb + 1) * QB], in_=den_r
            )
            nc.sync.dma_start(
                out=d["dbg_at"][:, qb * 2 * QB:(qb + 1) * 2 * QB],
                in_=at_sb.rearrange("p h n -> p (h n)"),
            )
            if qb == 0:
                nc.sync.dma_start(
                    out=d["dbg_p"][:, :], in_=p_sb_dbg.rearrange("p k n -> p (k n)")
                )
        return den_b

    def epilogue_co(qb, at_sb, den_b, co, po_pool=None, po_tag="po",
                    dma_split=False):
        po_pool = po_pool or ps_o
        po = po_pool.tile([P, QB], F32, name="po", tag=po_tag)
        nc.tensor.matmul(
            po, lhsT=wo8[:, :, co * P:(co + 1) * P], rhs=at_sb,
            start=True, stop=True, perf_mode=DR,
        )
        t1 = work.tile([P, QB], F32, name="t1", tag="t1")
        nc.vector.tensor_mul(t1, po, den_b)
        res = work.tile([P, QB], F32, name="res", tag="res", bufs=4)
        nc.vector.scalar_tensor_tensor(
            out=res, in0=t1, scalar=bo_eff[co],
            in1=x_sb[:, co, qb * QB:(qb + 1) * QB], op0=Alu.add, op1=Alu.add,
        )
        if dma_split:
            # tail-exposed final block: halve the transfer across both
            # HWDGE queues so the last bytes land ~1.5us earlier.
            HB = QB // 2
            for h, qeng in ((0, nc.sync), (1, nc.scalar)):
                qeng.dma_start(
                    out=out_d[co * P:(co + 1) * P,
                              qb * QB + h * HB:qb * QB + (h + 1) * HB],
                    in_=res[:, h * HB:(h + 1) * HB],
                )
        else:
            nc.sync.dma_start(
                out=out_d[co * P:(co + 1) * P, qb * QB:(qb + 1) * QB], in_=res
            )

    if DEBUG:
        nc.sync.dma_start(out=d["dbg_k"][:, :], in_=k_sb.rearrange("p h n -> p (h n)"))
        nc.sync.dma_start(out=d["dbg_q"][:, :], in_=q_sb.rearrange("p h n -> p (h n)"))
        nc.sync.dma_start(out=d["dbg_v"][:, :], in_=v_flat)

    pending = None
    pend_den = None
    for qb in range(NQB):
        p_sb = pblk.tile([P, NKT, QB], FP8, name="p_sb")
        dps = ps_d.tile([1, QB], F32, name="dps")
        aps = [
            ps_acc.tile([P, QB], F32, name="aps", tag="acc") for _ in range(2)
        ]
        at_sb = work.tile([P, 2, QB], FP8, name="at_sb", tag="at_sb", bufs=2)
        for j in range(NKP + 2):
            if j == 1 and pending is not None:
                pend_den = epilogue_a(*pending)
            if j == 2 and pending is not None:
                epilogue_co(pending[0], pending[3], pend_den, 0)
            if j == 3 and pending is not None:
                epilogue_co(pending[0], pending[3], pend_den, 1)
                pending = None
            if j < NKP:
                sps2 = ps.tile([P, 2 * QB], F32, name="sps2", tag="mm")
                for h2 in range(2):
                    kt = 2 * j + h2
                    nc.tensor.matmul(
                        sps2[:, h2 * QB:(h2 + 1) * QB],
                        lhsT=k_sb[:, :, kt * P:(kt + 1) * P],
                        rhs=q_sb[:, :, qb * QB:(qb + 1) * QB],
                        start=True, stop=True, perf_mode=DR,
                    )
                nc.scalar.activation(
                    p_sb[:, 2 * j:2 * j + 2, :], sps2, Act.Exp,
                    bias=esh_col, scale=SCALE,
                )
            if j >= 2:
                pj = j - 2
                pair = p_sb[:, 2 * pj:2 * pj + 2, :]
                for ch in range(2):
                    nc.tensor.matmul(
                        aps[ch],
                        lhsT=v_sb[:, 2 * pj:2 * pj + 2, ch * P:(ch + 1) * P],
                        rhs=pair,
                        start=(pj == 0), stop=(pj == NKP - 1),
                        perf_mode=DR, skip_group_check=True,
                    )
                nc.tensor.matmul(
                    dps, lhsT=ones8, rhs=pair,
                    start=(pj == 0), stop=(pj == NKP - 1),
                    perf_mode=DR, skip_group_check=True,
                )
        pending = (qb, dps, aps, at_sb, p_sb)
    pend_den = epilogue_a(*pending)
    epilogue_co(pending[0], pending[3], pend_den, 0, po_pool=ps, po_tag="mm",
                dma_split=True)
    epilogue_co(pending[0], pending[3], pend_den, 1, po_pool=ps, po_tag="mm",
                dma_split=True)

    for pool in (ps_o, ps_d, ps_acc, ps, work, pblk, small, stage, const):
        pool.release()


def build_program():
    global _NC
    if _NC is not None:
        return _NC
    nc = bacc.Bacc("TRN2", target_bir_lowering=False, debug=False,
                   num_devices=NCORES)
    d = {
        "x": nc.dram_tensor("x", [C, N], BF16, kind="ExternalInput"),
        "x8": nc.dram_tensor("x8", [C, N], FP8, kind="ExternalInput"),
        "wpack": nc.dram_tensor("wpack", [4 * C, C], BF16, kind="ExternalInput"),
        "bpack": nc.dram_tensor("bpack", [C, 6], F32, kind="ExternalInput"),
        "m1": nc.dram_tensor("m1", [P, G // 2], F32, kind="ExternalInput"),
        "m2": nc.dram_tensor("m2", [G // 2, P], F32, kind="ExternalInput"),
        "out": nc.dram_tensor("out", [C, NQ], F32, kind="ExternalOutput"),
    }
    if DEBUG:
        d.update({
            "dbg_k": nc.dram_tensor("dbg_k", [P, 2 * N], FP8, kind="ExternalOutput"),
            "dbg_q": nc.dram_tensor("dbg_q", [P, 2 * NQ], FP8, kind="ExternalOutput"),
            "dbg_v": nc.dram_tensor("dbg_v", [P, NKT * C], FP8, kind="ExternalOutput"),
            "dbg_p": nc.dram_tensor("dbg_p", [P, NKT * QB], FP8, kind="ExternalOutput"),
            "dbg_denr": nc.dram_tensor("dbg_denr", [1, NQ], F32, kind="ExternalOutput"),
            "dbg_at": nc.dram_tensor("dbg_at", [P, NQB * 2 * QB], FP8, kind="ExternalOutput"),
        })
    with tile.TileContext(nc) as tc:
        _body(tc, d)
    nc.compile()
    _NC = nc
    return nc


def make_in_maps(x, gamma, beta, wq, bq, wk, bk, wv, bv, wo, bo):
    import ml_dtypes

    f32c = lambda a: np.ascontiguousarray(np.asarray(a, dtype=np.float32))
    x = f32c(x)
    wpack = np.concatenate(
        [f32c(np.asarray(w, np.float32).T) for w in (wq, wk, wv, wo)], axis=0
    )
    bpack = np.stack(
        [f32c(v).reshape(C) for v in (bq, bk, bv, bo, gamma, beta)], axis=1
    )
    m1 = np.zeros((P, G // 2), np.float32)
    for g in range(G // 2):
        m1[8 * g:8 * g + 8, g] = 1.0
    base = {
        "wpack": np.ascontiguousarray(wpack.astype(ml_dtypes.bfloat16)),
        "bpack": np.ascontiguousarray(bpack),
        "m1": m1,
        "m2": np.ascontiguousarray(m1.T),
    }
    import ml_dtypes

    in_maps = []
    for core in range(NCORES):
        b, h = divmod(core, 2)
        xb = x[b].reshape(C, N)
        if h:
            xb = np.concatenate([xb[:, NQ:], xb[:, :NQ]], axis=1)
        in_maps.append({
            **base,
            "x": np.ascontiguousarray(xb.astype(ml_dtypes.bfloat16)),
            "x8": np.ascontiguousarray(xb.astype(ml_dtypes.float8_e4m3)),
        })
    return in_maps


def kernel(x, gamma, beta, wq, bq, wk, bk, wv, bv, wo, bo):
    global LAST_RESULTS
    from concourse.bass_utils import run_bass_kernel_spmd

    nc = build_program()
    in_maps = make_in_maps(x, gamma, beta, wq, bq, wk, bk, wv, bv, wo, bo)
    res = run_bass_kernel_spmd(nc, in_maps, core_ids=list(range(NCORES)))
    LAST_RESULTS = res
    out = np.empty((B, C, N), np.float32)
    for core in range(NCORES):
        b, h = divmod(core, 2)
        out[b][:, h * NQ:(h + 1) * NQ] = res.results[core]["out"]
    return out.reshape(B, C, H, W)



# revision 41
# speedup vs baseline: 1.0512x; 1.0512x over previous
"""AttnBlock (GroupNorm + single-head self-attention + residual) on 8 TRN2 cores.

Sharding: data-parallel over (batch b, query-half h) -> 8 shards. Each core
receives the full [C, N] image of its batch (columns rolled so that its own
query half always occupies columns 0:NQ), computes GroupNorm stats + K/V over
the whole image, Q over its half, and a flash-style attention in which scores
are produced directly transposed (S^T = K^T.T @ Q^T tiles) so softmax
normalization is done via a ones-vector matmul and no PE transposes of P are
needed.

All large matmuls (projections, S^T, PV, denominator, out-proj) run as fp8e4
DoubleRow matmuls: lhsT [128, 2, M] / rhs [128, 2, N] contract 256 deep in a
single instruction at ~2x bf16 FLOP rate. Weights are scaled x8 before the
fp8 cast (their entries are ~N(0, 1/16) and would hit e4m3 subnormals); the
scale is removed in the PSUM->SBUF cast. The softmax exp is shifted by -2
(exp(s/16 - 2)) so P fits e4m3's +-240 range; numerator and denominator share
the shift so the ratio is unchanged. exp runs on ACT in 2-key-tile batches
([128,1024] over a 2-bank PSUM tile) to amortize the per-call overhead, and
the denominator is a DoubleRow ones-matmul on the PE (accumulated in a
[1,512] PSUM bank), keeping the DVE free for casts and the epilogue.

Head/tail schedule (the attention loop itself runs at the PE's hardware pace,
~227ns per 512-free DoubleRow matmul):
- GroupNorm stats split DVE (13 bn_stats units) + ACT (3 units as Square /
  Identity passes with free-axis accum_out); a small leading x8 DMA chunk
  starts the stream early. All table sets contain identity/copy/square, so
  the sqrt set serves stats+dance and exp loads once, pinned into the
  projection phase by a fake bo_eff dependency on the dummy exp (the tile
  scheduler otherwise hoists it and the table sets ping-pong).
- The first K projection pair is issued right after wkt is scaled; the bias
  matvec chain rides the PE behind it. wo8 is pinned behind the dance via a
  zero-column bias dep (unpinned, its wstg wait stalls the ACT stats).
- A wstg-dependent warm-matmul block keeps HAM at full clock through the
  dance window; the final block's output DMA is halved across both HWDGE
  queues.
Hard-won constraints: Pool-engine (gpsimd) ucode tensor ops force library
swaps (UNLOAD/LOAD + multi-us drains) that stall the whole core — gpsimd may
only run partition_broadcast + DMA issues. Each DMA issue on the scalar
queue costs ~0.7us of ACT sequencer time. Per-queue DMA is ~90B/ns with
>=1KB contiguous runs, roughly half that for channel-split chunks.
"""

import os
import sys

import numpy as np

for _p in ("/opt/trn_rl_repo", "/root/.axon_site/_ro/trn_rl_repo"):
    if os.path.isdir(_p) and _p not in sys.path:
        sys.path.insert(0, _p)

import concourse.bass as bass  # noqa: E402
import concourse.tile as tile  # noqa: E402
from concourse import bacc, mybir  # noqa: E402
from concourse.masks import make_identity  # noqa: E402

# The agent image's antenv lacks axon_hooks; if BASS_TRACE is set in the
# environment, run_bass_kernel_spmd would crash importing it. Provide a stub
# (profiling degrades gracefully to "hook isn't registered").
try:
    import antenv.axon_hooks  # noqa: F401
except ImportError:
    import types as _types

    _m = _types.ModuleType("antenv.axon_hooks")
    _h = [None]
    _m.set_axon_ntff_profile_hook = lambda h: _h.__setitem__(0, h)
    _m.get_axon_ntff_profile_hook = lambda: _h[0]
    sys.modules["antenv.axon_hooks"] = _m

B, C, H, W = 4, 256, 64, 64
N = H * W  # 4096 pixels
NQ = N // 2  # 2048 queries per core
G = 32  # groups
CPG = C // G  # 8 channels per group
EPS = 1e-5
NCORES = 8
SCALE = float(C) ** -0.5  # 0.0625
ESHIFT = -3.0  # exp(s*SCALE + ESHIFT): data max logit 7.95 -> P <= ~141 < 240
WS = 8.0  # weight fp8 pre-scale (entries ~N(0,1/16) need lifting)

F32 = mybir.dt.float32
BF16 = mybir.dt.bfloat16
FP8 = mybir.dt.float8e4

QB = 512  # query block (free dim of S^T / PV matmuls)
NQB = NQ // QB  # 4 query blocks
NKT = N // 128  # 32 key tiles
NKP = NKT // 2  # 16 key-tile pairs (DoubleRow granularity)
NNB = N // QB  # 8 pixel blocks for K/V projections
P = 128

DEBUG = bool(int(os.environ.get("KDEBUG", "0")))

Act = mybir.ActivationFunctionType
Alu = mybir.AluOpType
Axis = mybir.AxisListType
DR = mybir.MatmulPerfMode.DoubleRow

_NC = None
LAST_RESULTS = None


def _body(tc, d):
    nc = tc.nc
    x_d = d["x"]
    out_d = d["out"]

    const = tc.alloc_tile_pool(name="const", bufs=1)
    stage = tc.alloc_tile_pool(name="stage", bufs=2)
    small = tc.alloc_tile_pool(name="small", bufs=1)
    pblk = tc.alloc_tile_pool(name="pblk", bufs=2)
    work = tc.alloc_tile_pool(name="work", bufs=2)
    # PSUM budget (8 banks): S-pair tiles 2x[P,1024] = 4, aps 2x[P,512] = 2,
    # dps [1,512] = 1, po [P,512] = 1.
    ps = tc.alloc_tile_pool(name="ps", bufs=2, space="PSUM")
    ps_acc = tc.alloc_tile_pool(name="ps_acc", bufs=2, space="PSUM")
    ps_d = tc.alloc_tile_pool(name="ps_d", bufs=1, space="PSUM")
    ps_o = tc.alloc_tile_pool(name="ps_o", bufs=1, space="PSUM")

    # ---- x in SBUF first: bf16 [128, 2(ch), 4096] (residual + stats) and
    # fp8 [128, 2, 4096] (matmul operand), both cast on host; chunked so
    # bn_stats overlaps the transfer ----
    x_sb = const.tile([P, 2, N], BF16)
    x8 = const.tile([P, 2, N], FP8)
    x_src = x_d.ap().rearrange("(h p) n -> p h n", p=P)
    x8_src = d["x8"].ap().rearrange("(h p) n -> p h n", p=P)

    # PE warm-up: keep the HAM activity monitor busy during the DMA/stats
    # window so projections and attention run at full clock from the start.
    # Memsets go FIRST so the warm matmuls aren't queued behind the bn_stats
    # chain in DVE program order (PE would idle for the whole DMA phase).
    wu_w = const.tile([P, P], BF16)
    nc.vector.memset(wu_w, 0.0)
    wu_x = const.tile([P, 2 * P], BF16)
    nc.vector.memset(wu_x, 0.0)
    wu_ps = ps.tile([P, QB], F32, name="wu_ps", tag="mm")

    def warm(n):
        for _ in range(n):
            nc.tensor.matmul(
                wu_ps[:, 0:2 * P], lhsT=wu_w, rhs=wu_x, start=True, stop=True
            )

    warm(2)

    # ---- head DMA: x8 (1MB, feeds stats AND all matmuls) goes FIRST in
    # 1024-col per-channel chunks: ch1 leads on both queues so the ACT
    # stats units and the DVE's first units have data earliest. ACT's own
    # HWDGE queue carries NO early DMA (each issue costs ~0.7us of ACT
    # sequencer time and starved the act-table loads in an earlier rev):
    # only the weight pack + one x_sb chunk ride scalar, the rest goes
    # sync + gpsimd (SWDGE).
    # Both-channel column chunks (1024B+ contiguous per partition transfers
    # ~90B/ns/queue; channel-split halves that). A small 512-col first
    # chunk lands ~1.5us earlier so the stats stream starts sooner.
    x8_chunks = [
        (nc.sync, 0, 1), (nc.scalar, 1, 2), (nc.gpsimd, 4, 6),
        (nc.sync, 2, 4), (nc.gpsimd, 6, 8),
    ]
    for q, u0, u1 in x8_chunks:
        sl = (slice(None), slice(None), slice(u0 * QB, u1 * QB))
        q.dma_start(out=x8[sl], in_=x8_src[sl])
    # all four weight matrices (bf16) in one DMA on the scalar queue (its
    # only early issue; needed at ~17us for the w_s scaling)
    wstg = stage.tile([P, 4, 2, C], BF16, name="wstg", tag="wstg")
    nc.scalar.dma_start(
        out=wstg, in_=d["wpack"].ap().rearrange("(w h p) co -> p w h co", p=P, h=2)
    )
    m1_sb = const.tile([P, G // 2], F32)
    nc.sync.dma_start(out=m1_sb, in_=d["m1"][:, :])
    m2_sb = const.tile([G // 2, P], F32)
    nc.sync.dma_start(out=m2_sb, in_=d["m2"][:, :])
    # packed biases/affine: rows (bq,bk,bv,bo,gamma,beta) -> per-channel cols
    bcols = const.tile([P, 2, 6], F32)
    nc.sync.dma_start(
        out=bcols, in_=d["bpack"].ap().rearrange("(h p) s -> p h s", p=P)
    )
    xsbq = [nc.sync, nc.sync, nc.gpsimd, nc.gpsimd]
    for c in range(4):
        sl = (slice(None), slice(None), slice(c * 2 * QB, (c + 1) * 2 * QB))
        xsbq[c].dma_start(out=x_sb[sl], in_=x_src[sl])

    # GroupNorm stats from the fp8 x (self-consistent: the projections
    # consume the same fp8 values; adds ~0.1% to the error budget). Split
    # across DVE (bn_stats: ch0 + ch1 units 0,1,2,3,7) and ACT (Square +
    # Identity passes with free-axis accumulate: ch1 units 4,5,6). Every
    # ACT table set contains identity+copy+square, so the sqrt set
    # (preloaded by the dummy sqrt below) serves the stats AND the dance;
    # the only other table load is the exp set, pinned into the projection
    # phase by the dummy exp's fake bo_eff dependency.
    eps16 = const.tile([G // 2, 1], F32)
    nc.vector.memset(eps16, EPS)
    one11 = const.tile([1, 1], F32)
    nc.vector.memset(one11, 1.0)
    warm11 = small.tile([1, 1], F32)
    nc.scalar.activation(warm11, one11, Act.Sqrt, scale=1.0)

    bn_st0 = small.tile([P, NNB, 6], F32, name="bn_st0")
    bn_st1b = small.tile([P, 5, 6], F32, name="bn_st1b")
    ACT_UNITS = (4, 5, 6)
    sq5 = small.tile([P, len(ACT_UNITS)], F32, name="sq5")
    sm5 = small.tile([P, len(ACT_UNITS)], F32, name="sm5")
    scr_bf = stage.tile([P, QB], BF16, name="scr_bf", tag="scr")
    for i, u in enumerate(ACT_UNITS):
        cols = slice(u * QB, (u + 1) * QB)
        nc.scalar.activation(
            scr_bf, x8[:, 1, cols], Act.Square, accum_out=sq5[:, i:i + 1]
        )
        nc.scalar.activation(
            scr_bf, x8[:, 1, cols], Act.Identity, accum_out=sm5[:, i:i + 1]
        )
    # DVE bn_stats in data-arrival order: small first chunk (u0), u1,
    # then gpsimd chunk (ch0 u4,u5), sync chunk (u2,u3), gpsimd tail.
    dve_units = [(1, 0, 0), (0, 0, None), (1, 1, 1), (0, 1, None),
                 (0, 4, None), (0, 5, None),
                 (1, 2, 2), (1, 3, 3), (0, 2, None), (0, 3, None),
                 (1, 7, 4), (0, 6, None), (0, 7, None)]
    for i, (ch, u, b1) in enumerate(dve_units):
        dst = bn_st0[:, u, :] if ch == 0 else bn_st1b[:, b1, :]
        nc.vector.bn_stats(out=dst, in_=x8[:, ch, u * QB:(u + 1) * QB])
        if i % 2 == 1:
            warm(6)

    # ---- constants ----
    # padded to 16B so the DoubleRow k-tile stride meets walrus' step%16==0.
    # Value 1.0 balances the at_sb 1/8 down-scale and the x8 wo lift:
    # po*den_r = (8/8)*wo@at_un / den = wo@at_un/den.
    ones8_pad = const.tile([P, 2, 16], FP8)
    nc.vector.memset(ones8_pad, 1.0)
    ones8 = ones8_pad[:, :, 0:1]
    esh_col = const.tile([P, 1], F32)
    nc.vector.memset(esh_col, ESHIFT)

    # ---- GroupNorm dance: per-channel (mean, E[x^2]) -> group reduce via
    # mask matmul -> rstd -> broadcast back via mask matmul -> a/b columns.
    # ch0 comes straight from bn_aggr; ch1 blends the 5 DVE units (2560
    # cols, weight 0.625) with the ACT accumulator sums (1536 cols).
    mvex = small.tile([P, 2, 2], F32, name="mvex")  # (mean, ex2) per ch-half
    mv0 = small.tile([P, 2], F32, name="mv0")
    nc.vector.bn_aggr(out=mv0, in_=bn_st0)
    mv1b = small.tile([P, 2], F32, name="mv1b")
    nc.vector.bn_aggr(out=mv1b, in_=bn_st1b)
    scol = small.tile([P, 2], F32, name="scol")
    nc.vector.tensor_reduce(
        out=scol[:, 0:1], in_=sm5, axis=Axis.X, op=Alu.add
    )
    nc.vector.tensor_reduce(
        out=scol[:, 1:2], in_=sq5, axis=Axis.X, op=Alu.add
    )
    tcol = small.tile([P, 6], F32, name="tcol")
    # ch0: mean, ex2 = var + mean^2
    nc.vector.tensor_copy(out=mvex[:, 0, 0:1], in_=mv0[:, 0:1])
    nc.vector.tensor_mul(tcol[:, 0:1], mv0[:, 0:1], mv0[:, 0:1])
    nc.vector.tensor_add(mvex[:, 0, 1:2], tcol[:, 0:1], mv0[:, 1:2])
    # ch1: mean = FB*mean_b + sum_id/4096
    FB = 2560.0 / 4096.0
    nc.vector.tensor_scalar_mul(tcol[:, 1:2], scol[:, 0:1], 1.0 / 4096.0)
    nc.vector.scalar_tensor_tensor(
        out=mvex[:, 1, 0:1], in0=mv1b[:, 0:1], scalar=FB, in1=tcol[:, 1:2],
        op0=Alu.mult, op1=Alu.add,
    )
    # ch1: ex2 = FB*(var_b + mean_b^2) + sum_sq/4096
    nc.vector.tensor_mul(tcol[:, 2:3], mv1b[:, 0:1], mv1b[:, 0:1])
    nc.vector.tensor_add(tcol[:, 3:4], mv1b[:, 1:2], tcol[:, 2:3])
    nc.vector.tensor_scalar_mul(tcol[:, 4:5], scol[:, 1:2], 1.0 / 4096.0)
    nc.vector.scalar_tensor_tensor(
        out=mvex[:, 1, 1:2], in0=tcol[:, 3:4], scalar=FB, in1=tcol[:, 4:5],
        op0=Alu.mult, op1=Alu.add,
    )
    # group sums over partitions: [16 groups, (mean0, ex20, mean1, ex21)]
    gsum = ps_o.tile([G // 2, 4], F32, name="gsum", tag="po")
    nc.tensor.matmul(
        gsum, lhsT=m1_sb, rhs=mvex.rearrange("p c k -> p (c k)"),
        start=True, stop=True,
    )
    warm(8)
    vals = small.tile([G // 2, 4], F32, name="vals")  # (rstd0, rstd1, m0, m1)
    gtmp = small.tile([G // 2, 4], F32, name="gtmp")
    gview = gsum.rearrange("g (c k) -> g c k", k=2)
    nc.vector.tensor_scalar_mul(
        vals.rearrange("g (c k) -> g c k", k=2)[:, 1, :], gview[:, :, 0],
        1.0 / CPG,
    )
    nc.vector.tensor_scalar_mul(gtmp[:, 0:2], gview[:, :, 1], 1.0 / CPG)
    nc.vector.tensor_mul(gtmp[:, 2:4], vals[:, 2:4], vals[:, 2:4])
    nc.vector.tensor_sub(gtmp[:, 0:2], gtmp[:, 0:2], gtmp[:, 2:4])
    # rstd = 1/sqrt(var + eps) (sqrt table preloaded; recip on DVE)
    nc.scalar.activation(gtmp[:, 2:4], gtmp[:, 0:2], Act.Sqrt, bias=eps16)
    nc.vector.reciprocal(vals[:, 0:2], gtmp[:, 2:4])
    # broadcast back to channels: [128, (rstd0, rstd1, m0, m1)]
    bc = ps_acc.tile([P, 4], F32, name="bc", tag="acc")
    nc.tensor.matmul(bc, lhsT=m2_sb, rhs=vals, start=True, stop=True)
    warm(8)
    # a = gamma * rstd; b = beta - mean * a; a8 = 8a (per channel cols)
    cols8 = small.tile([P, 8], F32, name="cols8")
    a_cols = [cols8[:, 0:1], cols8[:, 1:2]]
    b_cols = [cols8[:, 2:3], cols8[:, 3:4]]
    a8_cols = [cols8[:, 4:5], cols8[:, 5:6]]
    for ch in range(2):
        nc.vector.tensor_mul(a_cols[ch], bcols[:, ch, 4:5], bc[:, ch:ch + 1])
        nc.vector.tensor_mul(cols8[:, 6 + ch:7 + ch], bc[:, 2 + ch:3 + ch],
                             a_cols[ch])
        nc.vector.tensor_sub(b_cols[ch], bcols[:, ch, 5:6],
                             cols8[:, 6 + ch:7 + ch])
        nc.vector.tensor_scalar_mul(a8_cols[ch], a_cols[ch], WS)
    warm(8)
    # a zero column that data-depends on the dance: used to PIN late ACT
    # ops (wo8) behind the stats/dance — without a dep the scheduler
    # hoists them into the ACT stats stream where their wstg wait stalls it.
    zcol = small.tile([P, 1], F32, name="zcol")
    nc.vector.tensor_scalar_mul(zcol, a_cols[0], 0.0)
    # HAM filler through the dance: the dep-free warm matmuls all get
    # scheduled during the stats phase, so the PE goes quiet for the
    # ~17-22us dance window and HAM halves the clock right as the
    # projections start. These reads of the (just-landed) weight pack are
    # naturally pinned to that window.
    for i in range(28):
        nc.tensor.matmul(
            wu_ps[:, 0:C], lhsT=wu_w, rhs=wstg[:, i % 4, i % 2, :],
            start=True, stop=True,
        )
    # ...and a second filler block pinned to the END of the dance (wz
    # depends on cols8) so the density holds right up to the first
    # projection matmul and HAM never drops to K=4 there.
    wz = stage.tile([P, 2 * P], BF16, name="wz", tag="wz")
    nc.vector.tensor_scalar_mul(wz, wstg[:, 0, 0, 0:2 * P], a8_cols[0])
    for _ in range(10):
        nc.tensor.matmul(
            wu_ps[:, 0:2 * P], lhsT=wu_w, rhs=wz, start=True, stop=True
        )

    # scale wq/wk/wv rows by 8*a (per input channel) straight from the f32
    # staging into fp8 tiles. The x8 lift keeps the fp8 entries
    # (~N(0, a/16)) out of e4m3 subnormal range; the PSUM->SBUF casts
    # divide it back out.
    w_s = {}
    for wi, wname in ((0, "wqt"), (1, "wkt"), (2, "wvt")):
        w_s[wname] = const.tile([P, 2, C], FP8, name=f"{wname}_s")
    # wk first: it gates the longest projection stream (all on DVE — the
    # Pool engine's ucode tensor ops force library swaps that stall the
    # whole core).
    for wname, wi in (("wkt", 1), ("wqt", 0), ("wvt", 2)):
        for ci in range(2):
            nc.vector.tensor_scalar_mul(
                w_s[wname][:, ci, :], wstg[:, wi, ci, :], a8_cols[ci]
            )
    wo8 = const.tile([P, 2, C], FP8)

    # ---- projections (all DoubleRow fp8, contraction over full C=256) ----
    # K^T [C, N] fp8: psum[co,nb] = sum_ci wkt8[ci,co].T @ x8[ci, nb] (x8)
    # and q,k = psum/8 + bias. Casts alternate ACT / DVE (ACT is idle until
    # the attention loop's exp stream starts). The first K matmul pair is
    # issued the moment wkt is scaled; the tiny bias matvecs then ride the
    # PE behind it so their serial chain overlaps the projection stream.
    k_sb = const.tile([P, 2, N], FP8)
    q_sb = const.tile([P, 2, NQ], FP8)
    IWS = 1.0 / WS
    proj_ps = [(ps, "mm"), (ps_acc, "acc"), (ps, "mm"), (ps_o, "po"),
               (ps_acc, "acc")]
    _pi = [0]

    def proj_tile():
        pool, tag = proj_ps[_pi[0] % len(proj_ps)]
        _pi[0] += 1
        return pool.tile([P, QB], F32, name="pp", tag=tag)

    _ci = [0]

    def proj_cast(out, in_, bias):
        # only ACT/DVE can read PSUM; the 5-bank psum ring above keeps the
        # MM stream ~2.5 tiles ahead so the 2-engine cast pace (~325ns/tile)
        # doesn't stall the PE.
        i = _ci[0] % 2
        _ci[0] += 1
        if i == 0:
            nc.scalar.activation(out, in_, Act.Identity, bias=bias, scale=IWS)
        else:
            nc.vector.tensor_scalar(
                out=out, in0=in_, scalar1=IWS, scalar2=bias, op0=Alu.mult,
                op1=Alu.add,
            )

    # first K pair (block 0) — only these two proj psums may be in flight
    # before the matvec chain (deeper would deadlock through be_*'s ring
    # reuse of the same PSUM tags).
    pk0 = [proj_tile() for _ in range(2)]
    for co in range(2):
        nc.tensor.matmul(
            pk0[co], lhsT=w_s["wkt"][:, :, co * P:(co + 1) * P],
            rhs=x8[:, :, 0:QB], start=True, stop=True, perf_mode=DR,
        )

    # projection bias columns: be = W b + bias (bf16 matvecs off the staged
    # weights; psum ring across three pools so the chain pipelines)
    b_bf = small.tile([P, 2], BF16, name="b_bf")
    for ch in range(2):
        nc.vector.tensor_copy(out=b_bf[:, ch:ch + 1], in_=b_cols[ch])
    _mvi = [0]
    mv_pools = [(ps_o, "po"), (ps_acc, "acc"), (ps, "mm")]

    def matvec_bias(wi, rhs_cols, bias_idx, out_name, out_dt=F32):
        outs = []
        for co in range(2):
            pool, tag = mv_pools[_mvi[0] % 3]
            _mvi[0] += 1
            pe = pool.tile([P, 1], F32, name="pe_mv", tag=tag)
            for ci in range(2):
                nc.tensor.matmul(
                    pe, lhsT=wstg[:, wi, ci, co * P:(co + 1) * P],
                    rhs=rhs_cols[ci], start=(ci == 0), stop=(ci == 1),
                )
            t = small.tile([P, 1], out_dt, name=f"{out_name}_{co}")
            nc.scalar.activation(
                t, pe, Act.Identity, bias=bcols[:, co, bias_idx:bias_idx + 1],
                scale=1.0,
            )
            outs.append(t)
        return outs

    bcol_list = [b_bf[:, 0:1], b_bf[:, 1:2]]
    be_q = matvec_bias(0, bcol_list, 0, "be_q")
    be_k = matvec_bias(1, bcol_list, 1, "be_k")
    vbv = matvec_bias(2, bcol_list, 2, "vbv", out_dt=BF16)
    bo_eff = matvec_bias(3, vbv, 3, "bo_eff")
    # dummy exp: pulls the exp table load into the projection phase. The
    # scale AP is a fake data dependency on bo_eff — without it the tile
    # scheduler hoists this exp into the stats phase and the sqrt/exp
    # table sets ping-pong (4 extra 1.5us table loads).
    nc.scalar.activation(warm11, one11, Act.Exp, scale=bo_eff[0][0:1, 0:1])
    for co in range(2):
        proj_cast(k_sb[:, co, 0:QB], pk0[co], be_k[co])
    # wo8 = 8*wo in fp8 (no GroupNorm folding on the out-proj); the zcol
    # bias pins these behind the dance (first consumer is qb0's epilogue,
    # ~20us later).
    for ch in range(2):
        nc.scalar.activation(wo8[:, ch, :], wstg[:, 3, ch, :], Act.Identity,
                             bias=zcol, scale=WS)

    for nb in range(NNB):
        for co in range(2):
            if nb < NQB:
                pq = proj_tile()
                nc.tensor.matmul(
                    pq, lhsT=w_s["wqt"][:, :, co * P:(co + 1) * P],
                    rhs=x8[:, :, nb * QB:(nb + 1) * QB],
                    start=True, stop=True, perf_mode=DR,
                )
                proj_cast(q_sb[:, co, nb * QB:(nb + 1) * QB], pq, be_q[co])
            if nb >= 1:
                pk = proj_tile()
                nc.tensor.matmul(
                    pk, lhsT=w_s["wkt"][:, :, co * P:(co + 1) * P],
                    rhs=x8[:, :, nb * QB:(nb + 1) * QB],
                    start=True, stop=True, perf_mode=DR,
                )
                proj_cast(k_sb[:, co, nb * QB:(nb + 1) * QB], pk, be_k[co])

    # V [N, C] fp8 (bias folded into bo_eff): psum[nt] = x8_tile.T @ wvt8
    v_sb = const.tile([P, NKT, C], FP8)
    v_flat = v_sb.rearrange("p k c -> p (k c)")
    zero_col = const.tile([P, 1], F32)
    nc.vector.memset(zero_col, 0.0)
    for nt in range(0, NKT, 2):
        pv = proj_tile()
        for n2 in range(2):
            nc.tensor.matmul(
                pv[:, n2 * C:(n2 + 1) * C],
                lhsT=x8[:, :, (nt + n2) * P:(nt + n2 + 1) * P],
                rhs=w_s["wvt"][:, :, :],
                start=True, stop=True, perf_mode=DR,
            )
        proj_cast(v_flat[:, nt * C:(nt + 2) * C], pv, zero_col)

    # ---- attention, per query block; DoubleRow over key-tile pairs with a
    # batched exp (one ACT call per pair reading a 2-bank PSUM tile) and the
    # denominator as a DoubleRow ones-matmul on the PE. The softmax division
    # is commuted through the out-projection: out = (wo8 @ PV) * (1/(8*den))
    # + bo_eff + x, deferred one qb so nothing waits on the reciprocal.
    def epilogue_a(qb, dps, aps, at_sb, p_sb_dbg=None):
        # casts first: they release the PV accumulator banks immediately.
        # 1/8 keeps the heavy-tailed PV numerator inside fp8's +-240.
        nc.vector.tensor_scalar_mul(at_sb[:, 0, :], aps[0], 1.0 / 8.0)
        nc.vector.tensor_scalar_mul(at_sb[:, 1, :], aps[1], 1.0 / 8.0)
        den_r = work.tile([1, QB], F32, name="den_r", tag="den_r")
        nc.vector.reciprocal_approx_fast(out=den_r, in_=dps)
        den_b = work.tile([P, QB], F32, name="den_b", tag="den_b", bufs=2)
        nc.gpsimd.partition_broadcast(den_b, den_r)
        if DEBUG:
            nc.sync.dma_start(
                out=d["dbg_denr"][:, qb * QB:(qb + 1) * QB], in_=den_r
            )
            nc.sync.dma_start(
                out=d["dbg_at"][:, qb * 2 * QB:(qb + 1) * 2 * QB],
                in_=at_sb.rearrange("p h n -> p (h n)"),
            )
            if qb == 0:
                nc.sync.dma_start(
                    out=d["dbg_p"][:, :], in_=p_sb_dbg.rearrange("p k n -> p (k n)")
                )
        return den_b

    def epilogue_co(qb, at_sb, den_b, co, po_pool=None, po_tag="po",
                    dma_split=False):
        po_pool = po_pool or ps_o
        po = po_pool.tile([P, QB], F32, name="po", tag=po_tag)
        nc.tensor.matmul(
            po, lhsT=wo8[:, :, co * P:(co + 1) * P], rhs=at_sb,
            start=True, stop=True, perf_mode=DR,
        )
        t1 = work.tile([P, QB], F32, name="t1", tag="t1")
        nc.vector.tensor_mul(t1, po, den_b)
        res = work.tile([P, QB], F32, name="res", tag="res", bufs=4)
        nc.vector.scalar_tensor_tensor(
            out=res, in0=t1, scalar=bo_eff[co],
            in1=x_sb[:, co, qb * QB:(qb + 1) * QB], op0=Alu.add, op1=Alu.add,
        )
        if dma_split:
            # tail-exposed final block: halve the transfer across both
            # HWDGE queues so the last bytes land ~1.5us earlier.
            HB = QB // 2
            for h, qeng in ((0, nc.sync), (1, nc.scalar)):
                qeng.dma_start(
                    out=out_d[co * P:(co + 1) * P,
                              qb * QB + h * HB:qb * QB + (h + 1) * HB],
                    in_=res[:, h * HB:(h + 1) * HB],
                )
        else:
            nc.sync.dma_start(
                out=out_d[co * P:(co + 1) * P, qb * QB:(qb + 1) * QB], in_=res
            )

    if DEBUG:
        nc.sync.dma_start(out=d["dbg_k"][:, :], in_=k_sb.rearrange("p h n -> p (h n)"))
        nc.sync.dma_start(out=d["dbg_q"][:, :], in_=q_sb.rearrange("p h n -> p (h n)"))
        nc.sync.dma_start(out=d["dbg_v"][:, :], in_=v_flat)

    pending = None
    pend_den = None
    for qb in range(NQB):
        p_sb = pblk.tile([P, NKT, QB], FP8, name="p_sb")
        dps = ps_d.tile([1, QB], F32, name="dps")
        aps = [
            ps_acc.tile([P, QB], F32, name="aps", tag="acc") for _ in range(2)
        ]
        at_sb = work.tile([P, 2, QB], FP8, name="at_sb", tag="at_sb", bufs=2)
        for j in range(NKP + 2):
            if j == 1 and pending is not None:
                pend_den = epilogue_a(*pending)
            if j == 2 and pending is not None:
                epilogue_co(pending[0], pending[3], pend_den, 0)
            if j == 3 and pending is not None:
                epilogue_co(pending[0], pending[3], pend_den, 1)
                pending = None
            if j < NKP:
                sps2 = ps.tile([P, 2 * QB], F32, name="sps2", tag="mm")
                for h2 in range(2):
                    kt = 2 * j + h2
                    nc.tensor.matmul(
                        sps2[:, h2 * QB:(h2 + 1) * QB],
                        lhsT=k_sb[:, :, kt * P:(kt + 1) * P],
                        rhs=q_sb[:, :, qb * QB:(qb + 1) * QB],
                        start=True, stop=True, perf_mode=DR,
                    )
                nc.scalar.activation(
                    p_sb[:, 2 * j:2 * j + 2, :], sps2, Act.Exp,
                    bias=esh_col, scale=SCALE,
                )
            if j >= 2:
                pj = j - 2
                pair = p_sb[:, 2 * pj:2 * pj + 2, :]
                for ch in range(2):
                    nc.tensor.matmul(
                        aps[ch],
                        lhsT=v_sb[:, 2 * pj:2 * pj + 2, ch * P:(ch + 1) * P],
                        rhs=pair,
                        start=(pj == 0), stop=(pj == NKP - 1),
                        perf_mode=DR, skip_group_check=True,
                    )
                nc.tensor.matmul(
                    dps, lhsT=ones8, rhs=pair,
                    start=(pj == 0), stop=(pj == NKP - 1),
                    perf_mode=DR, skip_group_check=True,
                )
        pending = (qb, dps, aps, at_sb, p_sb)
    pend_den = epilogue_a(*pending)
    epilogue_co(pending[0], pending[3], pend_den, 0, po_pool=ps, po_tag="mm",
                dma_split=True)
    epilogue_co(pending[0], pending[3], pend_den, 1, po_pool=ps, po_tag="mm",
                dma_split=True)

    for pool in (ps_o, ps_d, ps_acc, ps, work, pblk, small, stage, const):
        pool.release()


def build_program():
    global _NC
    if _NC is not None:
        return _NC
    nc = bacc.Bacc("TRN2", target_bir_lowering=False, debug=False,
                   num_devices=NCORES)
    d = {
        "x": nc.dram_tensor("x", [C, N], BF16, kind="ExternalInput"),
        "x8": nc.dram_tensor("x8", [C, N], FP8, kind="ExternalInput"),
        "wpack": nc.dram_tensor("wpack", [4 * C, C], BF16, kind="ExternalInput"),
        "bpack": nc.dram_tensor("bpack", [C, 6], F32, kind="ExternalInput"),
        "m1": nc.dram_tensor("m1", [P, G // 2], F32, kind="ExternalInput"),
        "m2": nc.dram_tensor("m2", [G // 2, P], F32, kind="ExternalInput"),
        "out": nc.dram_tensor("out", [C, NQ], F32, kind="ExternalOutput"),
    }
    if DEBUG:
        d.update({
            "dbg_k": nc.dram_tensor("dbg_k", [P, 2 * N], FP8, kind="ExternalOutput"),
            "dbg_q": nc.dram_tensor("dbg_q", [P, 2 * NQ], FP8, kind="ExternalOutput"),
            "dbg_v": nc.dram_tensor("dbg_v", [P, NKT * C], FP8, kind="ExternalOutput"),
            "dbg_p": nc.dram_tensor("dbg_p", [P, NKT * QB], FP8, kind="ExternalOutput"),
            "dbg_denr": nc.dram_tensor("dbg_denr", [1, NQ], F32, kind="ExternalOutput"),
            "dbg_at": nc.dram_tensor("dbg_at", [P, NQB * 2 * QB], FP8, kind="ExternalOutput"),
        })
    with tile.TileContext(nc) as tc:
        _body(tc, d)
    nc.compile()
    _NC = nc
    return nc


def make_in_maps(x, gamma, beta, wq, bq, wk, bk, wv, bv, wo, bo):
    import ml_dtypes

    f32c = lambda a: np.ascontiguousarray(np.asarray(a, dtype=np.float32))
    x = f32c(x)
    wpack = np.concatenate(
        [f32c(np.asarray(w, np.float32).T) for w in (wq, wk, wv, wo)], axis=0
    )
    bpack = np.stack(
        [f32c(v).reshape(C) for v in (bq, bk, bv, bo, gamma, beta)], axis=1
    )
    m1 = np.zeros((P, G // 2), np.float32)
    for g in range(G // 2):
        m1[8 * g:8 * g + 8, g] = 1.0
    base = {
        "wpack": np.ascontiguousarray(wpack.astype(ml_dtypes.bfloat16)),
        "bpack": np.ascontiguousarray(bpack),
        "m1": m1,
        "m2": np.ascontiguousarray(m1.T),
    }
    import ml_dtypes

    in_maps = []
    for core in range(NCORES):
        b, h = divmod(core, 2)
        xb = x[b].reshape(C, N)
        if h:
            xb = np.concatenate([xb[:, NQ:], xb[:, :NQ]], axis=1)
        in_maps.append({
            **base,
            "x": np.ascontiguousarray(xb.astype(ml_dtypes.bfloat16)),
            "x8": np.ascontiguousarray(xb.astype(ml_dtypes.float8_e4m3)),
        })
    return in_maps


def kernel(x, gamma, beta, wq, bq, wk, bk, wv, bv, wo, bo):
    global LAST_RESULTS
    from concourse.bass_utils import run_bass_kernel_spmd

    nc = build_program()
    in_maps = make_in_maps(x, gamma, beta, wq, bq, wk, bk, wv, bv, wo, bo)
    res = run_bass_kernel_spmd(nc, in_maps, core_ids=list(range(NCORES)))
    LAST_RESULTS = res
    out = np.empty((B, C, N), np.float32)
    for core in range(NCORES):
        b, h = divmod(core, 2)
        out[b][:, h * NQ:(h + 1) * NQ] = res.results[core]["out"]
    return out.reshape(B, C, H, W)



# revision 44
# speedup vs baseline: 1.2107x; 1.1517x over previous
"""AttnBlock (GroupNorm + single-head self-attention + residual) on 8 TRN2 cores.

Sharding: data-parallel over (batch b, query-half h) -> 8 shards. Each core
receives the full [C, N] image of its batch (columns rolled so that its own
query half always occupies columns 0:NQ), computes GroupNorm stats + K/V over
the whole image, Q over its half, and a flash-style attention in which scores
are produced directly transposed (S^T = K^T.T @ Q^T tiles) so softmax
normalization is done via a ones-vector matmul and no PE transposes of P are
needed.

All large matmuls (projections, S^T, PV, denominator, out-proj) run as fp8e4
DoubleRow matmuls: lhsT [128, 2, M] / rhs [128, 2, N] contract 256 deep in a
single instruction at ~2x bf16 FLOP rate. Weights are scaled x8 before the
fp8 cast (their entries are ~N(0, 1/16) and would hit e4m3 subnormals); the
scale is removed in the PSUM->SBUF cast. The softmax exp is shifted by -2
(exp(s/16 - 2)) so P fits e4m3's +-240 range; numerator and denominator share
the shift so the ratio is unchanged. exp runs on ACT in 2-key-tile batches
([128,1024] over a 2-bank PSUM tile) to amortize the per-call overhead, and
the denominator is a DoubleRow ones-matmul on the PE (accumulated in a
[1,512] PSUM bank), keeping the DVE free for casts and the epilogue.

Head/tail schedule (the attention loop itself runs at the PE's hardware pace,
~227ns per 512-free DoubleRow matmul):
- GroupNorm stats split DVE (13 bn_stats units) + ACT (3 units as Square /
  Identity passes with free-axis accum_out); a small leading x8 DMA chunk
  starts the stream early. All table sets contain identity/copy/square, so
  the sqrt set serves stats+dance and exp loads once, pinned into the
  projection phase by a fake bo_eff dependency on the dummy exp (the tile
  scheduler otherwise hoists it and the table sets ping-pong).
- The first K projection pair is issued right after wkt is scaled; the bias
  matvec chain rides the PE behind it. wo8 is pinned behind the dance via a
  zero-column bias dep (unpinned, its wstg wait stalls the ACT stats).
- A wstg-dependent warm-matmul block keeps HAM at full clock through the
  dance window; the final block's output DMA is halved across both HWDGE
  queues.
Hard-won constraints: Pool-engine (gpsimd) ucode tensor ops force library
swaps (UNLOAD/LOAD + multi-us drains) that stall the whole core — gpsimd may
only run partition_broadcast + DMA issues. Each DMA issue on the scalar
queue costs ~0.7us of ACT sequencer time. Per-queue DMA is ~90B/ns with
>=1KB contiguous runs, roughly half that for channel-split chunks.
"""

import os
import sys

import numpy as np

for _p in ("/opt/trn_rl_repo", "/root/.axon_site/_ro/trn_rl_repo"):
    if os.path.isdir(_p) and _p not in sys.path:
        sys.path.insert(0, _p)

import concourse.bass as bass  # noqa: E402
import concourse.tile as tile  # noqa: E402
from concourse import bacc, mybir  # noqa: E402
from concourse.masks import make_identity  # noqa: E402

# The agent image's antenv lacks axon_hooks; if BASS_TRACE is set in the
# environment, run_bass_kernel_spmd would crash importing it. Provide a stub
# (profiling degrades gracefully to "hook isn't registered").
try:
    import antenv.axon_hooks  # noqa: F401
except ImportError:
    import types as _types

    _m = _types.ModuleType("antenv.axon_hooks")
    _h = [None]
    _m.set_axon_ntff_profile_hook = lambda h: _h.__setitem__(0, h)
    _m.get_axon_ntff_profile_hook = lambda: _h[0]
    sys.modules["antenv.axon_hooks"] = _m

B, C, H, W = 4, 256, 64, 64
N = H * W  # 4096 pixels
NQ = N // 2  # 2048 queries per core
G = 32  # groups
CPG = C // G  # 8 channels per group
EPS = 1e-5
NCORES = 8
SCALE = float(C) ** -0.5  # 0.0625
ESHIFT = -3.0  # exp(s*SCALE + ESHIFT): data max logit 7.95 -> P <= ~141 < 240
WS = 8.0  # weight fp8 pre-scale (entries ~N(0,1/16) need lifting)

F32 = mybir.dt.float32
BF16 = mybir.dt.bfloat16
FP8 = mybir.dt.float8e4

QB = 512  # query block (free dim of S^T / PV matmuls)
NQB = NQ // QB  # 4 query blocks
NKT = N // 128  # 32 key tiles
NKP = NKT // 2  # 16 key-tile pairs (DoubleRow granularity)
NNB = N // QB  # 8 pixel blocks for K/V projections
P = 128

DEBUG = bool(int(os.environ.get("KDEBUG", "0")))

Act = mybir.ActivationFunctionType
Alu = mybir.AluOpType
Axis = mybir.AxisListType
DR = mybir.MatmulPerfMode.DoubleRow

_NC = None
LAST_RESULTS = None


def _body(tc, d):
    nc = tc.nc
    x_d = d["x"]
    out_d = d["out"]

    const = tc.alloc_tile_pool(name="const", bufs=1)
    stage = tc.alloc_tile_pool(name="stage", bufs=2)
    small = tc.alloc_tile_pool(name="small", bufs=1)
    pblk = tc.alloc_tile_pool(name="pblk", bufs=2)
    work = tc.alloc_tile_pool(name="work", bufs=2)
    # PSUM budget (8 banks): S-pair tiles 2x[P,1024] = 4, aps 2x[P,512] = 2,
    # dps [1,512] = 1, po [P,512] = 1.
    ps = tc.alloc_tile_pool(name="ps", bufs=2, space="PSUM")
    ps_acc = tc.alloc_tile_pool(name="ps_acc", bufs=2, space="PSUM")
    ps_d = tc.alloc_tile_pool(name="ps_d", bufs=1, space="PSUM")
    ps_o = tc.alloc_tile_pool(name="ps_o", bufs=1, space="PSUM")

    # ---- x in SBUF first: bf16 [128, 2(ch), 4096] (residual + stats) and
    # fp8 [128, 2, 4096] (matmul operand), both cast on host; chunked so
    # bn_stats overlaps the transfer ----
    x_sb = const.tile([P, 2, N], BF16)
    x8 = const.tile([P, 2, N], FP8)
    x_src = x_d.ap().rearrange("(h p) n -> p h n", p=P)
    x8_src = d["x8"].ap().rearrange("(h p) n -> p h n", p=P)

    # PE warm-up: keep the HAM activity monitor busy during the DMA/stats
    # window so projections and attention run at full clock from the start.
    # Memsets go FIRST so the warm matmuls aren't queued behind the bn_stats
    # chain in DVE program order (PE would idle for the whole DMA phase).
    wu_w = const.tile([P, P], BF16)
    nc.vector.memset(wu_w, 0.0)
    wu_x = const.tile([P, 2 * P], BF16)
    nc.vector.memset(wu_x, 0.0)
    wu_ps = ps.tile([P, QB], F32, name="wu_ps", tag="mm")

    def warm(n):
        for _ in range(n):
            nc.tensor.matmul(
                wu_ps[:, 0:2 * P], lhsT=wu_w, rhs=wu_x, start=True, stop=True
            )

    warm(2)

    # ---- head DMA: x8 (1MB, feeds stats AND all matmuls) goes FIRST in
    # 1024-col per-channel chunks: ch1 leads on both queues so the ACT
    # stats units and the DVE's first units have data earliest. ACT's own
    # HWDGE queue carries NO early DMA (each issue costs ~0.7us of ACT
    # sequencer time and starved the act-table loads in an earlier rev):
    # only the weight pack + one x_sb chunk ride scalar, the rest goes
    # sync + gpsimd (SWDGE).
    # Both-channel column chunks (1024B+ contiguous per partition transfers
    # ~90B/ns/queue; channel-split halves that). A small 512-col first
    # chunk lands ~1.5us earlier so the stats stream starts sooner.
    x8_chunks = [
        (nc.sync, 0, 1), (nc.scalar, 1, 2), (nc.gpsimd, 4, 6),
        (nc.sync, 2, 4), (nc.gpsimd, 6, 8),
    ]
    for q, u0, u1 in x8_chunks:
        sl = (slice(None), slice(None), slice(u0 * QB, u1 * QB))
        q.dma_start(out=x8[sl], in_=x8_src[sl])
    # all four weight matrices (bf16) in one DMA on the scalar queue (its
    # only early issue; needed at ~17us for the w_s scaling)
    wstg = stage.tile([P, 4, 2, C], BF16, name="wstg", tag="wstg")
    nc.scalar.dma_start(
        out=wstg, in_=d["wpack"].ap().rearrange("(w h p) co -> p w h co", p=P, h=2)
    )
    m1_sb = const.tile([P, G // 2], F32)
    nc.sync.dma_start(out=m1_sb, in_=d["m1"][:, :])
    m2_sb = const.tile([G // 2, P], F32)
    nc.sync.dma_start(out=m2_sb, in_=d["m2"][:, :])
    # packed biases/affine: rows (bq,bk,bv,bo,gamma,beta) -> per-channel cols
    bcols = const.tile([P, 2, 6], F32)
    nc.sync.dma_start(
        out=bcols, in_=d["bpack"].ap().rearrange("(h p) s -> p h s", p=P)
    )
    xsbq = [nc.sync, nc.sync, nc.gpsimd, nc.gpsimd]
    for c in range(4):
        sl = (slice(None), slice(None), slice(c * 2 * QB, (c + 1) * 2 * QB))
        xsbq[c].dma_start(out=x_sb[sl], in_=x_src[sl])

    # GroupNorm stats from the fp8 x (self-consistent: the projections
    # consume the same fp8 values; adds ~0.1% to the error budget). Split
    # across DVE (bn_stats: ch0 + ch1 units 0,1,2,3,7) and ACT (Square +
    # Identity passes with free-axis accumulate: ch1 units 4,5,6). Every
    # ACT table set contains identity+copy+square, so the sqrt set
    # (preloaded by the dummy sqrt below) serves the stats AND the dance;
    # the only other table load is the exp set, pinned into the projection
    # phase by the dummy exp's fake bo_eff dependency.
    eps16 = const.tile([G // 2, 1], F32)
    nc.vector.memset(eps16, EPS)
    one11 = const.tile([1, 1], F32)
    nc.vector.memset(one11, 1.0)
    warm11 = small.tile([1, 1], F32)
    nc.scalar.activation(warm11, one11, Act.Sqrt, scale=1.0)

    bn_st0 = small.tile([P, NNB, 6], F32, name="bn_st0")
    bn_st1b = small.tile([P, 5, 6], F32, name="bn_st1b")
    ACT_UNITS = (4, 5, 6)
    sq5 = small.tile([P, len(ACT_UNITS)], F32, name="sq5")
    sm5 = small.tile([P, len(ACT_UNITS)], F32, name="sm5")
    scr_bf = stage.tile([P, QB], BF16, name="scr_bf", tag="scr")
    for i, u in enumerate(ACT_UNITS):
        cols = slice(u * QB, (u + 1) * QB)
        nc.scalar.activation(
            scr_bf, x8[:, 1, cols], Act.Square, accum_out=sq5[:, i:i + 1]
        )
        nc.scalar.activation(
            scr_bf, x8[:, 1, cols], Act.Identity, accum_out=sm5[:, i:i + 1]
        )
    # DVE bn_stats in data-arrival order: small first chunk (u0), u1,
    # then gpsimd chunk (ch0 u4,u5), sync chunk (u2,u3), gpsimd tail.
    dve_units = [(1, 0, 0), (0, 0, None), (1, 1, 1), (0, 1, None),
                 (0, 4, None), (0, 5, None),
                 (1, 2, 2), (1, 3, 3), (0, 2, None), (0, 3, None),
                 (1, 7, 4), (0, 6, None), (0, 7, None)]
    for i, (ch, u, b1) in enumerate(dve_units):
        dst = bn_st0[:, u, :] if ch == 0 else bn_st1b[:, b1, :]
        nc.vector.bn_stats(out=dst, in_=x8[:, ch, u * QB:(u + 1) * QB])
        if i % 2 == 1:
            warm(6)

    # ---- constants ----
    # padded to 16B so the DoubleRow k-tile stride meets walrus' step%16==0.
    # Value 1.0 balances the at_sb 1/8 down-scale and the x8 wo lift:
    # po*den_r = (8/8)*wo@at_un / den = wo@at_un/den.
    ones8_pad = const.tile([P, 2, 16], FP8)
    nc.vector.memset(ones8_pad, 1.0)
    ones8 = ones8_pad[:, :, 0:1]
    esh_col = const.tile([P, 1], F32)
    nc.vector.memset(esh_col, ESHIFT)

    # ---- GroupNorm dance: per-channel (mean, E[x^2]) -> group reduce via
    # mask matmul -> rstd -> broadcast back via mask matmul -> a/b columns.
    # ch0 comes straight from bn_aggr; ch1 blends the 5 DVE units (2560
    # cols, weight 0.625) with the ACT accumulator sums (1536 cols).
    mvex = small.tile([P, 2, 2], F32, name="mvex")  # (mean, ex2) per ch-half
    mv0 = small.tile([P, 2], F32, name="mv0")
    nc.vector.bn_aggr(out=mv0, in_=bn_st0)
    mv1b = small.tile([P, 2], F32, name="mv1b")
    nc.vector.bn_aggr(out=mv1b, in_=bn_st1b)
    scol = small.tile([P, 2], F32, name="scol")
    nc.vector.tensor_reduce(
        out=scol[:, 0:1], in_=sm5, axis=Axis.X, op=Alu.add
    )
    nc.vector.tensor_reduce(
        out=scol[:, 1:2], in_=sq5, axis=Axis.X, op=Alu.add
    )
    tcol = small.tile([P, 6], F32, name="tcol")
    # ch0: mean, ex2 = var + mean^2
    nc.vector.tensor_copy(out=mvex[:, 0, 0:1], in_=mv0[:, 0:1])
    nc.vector.tensor_mul(tcol[:, 0:1], mv0[:, 0:1], mv0[:, 0:1])
    nc.vector.tensor_add(mvex[:, 0, 1:2], tcol[:, 0:1], mv0[:, 1:2])
    # ch1: mean = FB*mean_b + sum_id/4096
    FB = 2560.0 / 4096.0
    nc.vector.tensor_scalar_mul(tcol[:, 1:2], scol[:, 0:1], 1.0 / 4096.0)
    nc.vector.scalar_tensor_tensor(
        out=mvex[:, 1, 0:1], in0=mv1b[:, 0:1], scalar=FB, in1=tcol[:, 1:2],
        op0=Alu.mult, op1=Alu.add,
    )
    # ch1: ex2 = FB*(var_b + mean_b^2) + sum_sq/4096
    nc.vector.tensor_mul(tcol[:, 2:3], mv1b[:, 0:1], mv1b[:, 0:1])
    nc.vector.tensor_add(tcol[:, 3:4], mv1b[:, 1:2], tcol[:, 2:3])
    nc.vector.tensor_scalar_mul(tcol[:, 4:5], scol[:, 1:2], 1.0 / 4096.0)
    nc.vector.scalar_tensor_tensor(
        out=mvex[:, 1, 1:2], in0=tcol[:, 3:4], scalar=FB, in1=tcol[:, 4:5],
        op0=Alu.mult, op1=Alu.add,
    )
    # group sums over partitions: [16 groups, (mean0, ex20, mean1, ex21)]
    gsum = ps_o.tile([G // 2, 4], F32, name="gsum", tag="po")
    nc.tensor.matmul(
        gsum, lhsT=m1_sb, rhs=mvex.rearrange("p c k -> p (c k)"),
        start=True, stop=True,
    )
    warm(8)
    vals = small.tile([G // 2, 4], F32, name="vals")  # (rstd0, rstd1, m0, m1)
    gtmp = small.tile([G // 2, 4], F32, name="gtmp")
    gview = gsum.rearrange("g (c k) -> g c k", k=2)
    nc.vector.tensor_scalar_mul(
        vals.rearrange("g (c k) -> g c k", k=2)[:, 1, :], gview[:, :, 0],
        1.0 / CPG,
    )
    nc.vector.tensor_scalar_mul(gtmp[:, 0:2], gview[:, :, 1], 1.0 / CPG)
    nc.vector.tensor_mul(gtmp[:, 2:4], vals[:, 2:4], vals[:, 2:4])
    nc.vector.tensor_sub(gtmp[:, 0:2], gtmp[:, 0:2], gtmp[:, 2:4])
    # rstd = 1/sqrt(var + eps) (sqrt table preloaded; recip on DVE)
    nc.scalar.activation(gtmp[:, 2:4], gtmp[:, 0:2], Act.Sqrt, bias=eps16)
    nc.vector.reciprocal(vals[:, 0:2], gtmp[:, 2:4])
    # broadcast back to channels: [128, (rstd0, rstd1, m0, m1)]
    bc = ps_acc.tile([P, 4], F32, name="bc", tag="acc")
    nc.tensor.matmul(bc, lhsT=m2_sb, rhs=vals, start=True, stop=True)
    warm(8)
    # a = gamma * rstd; b = beta - mean * a; a8 = 8a (per channel cols)
    cols8 = small.tile([P, 8], F32, name="cols8")
    a_cols = [cols8[:, 0:1], cols8[:, 1:2]]
    b_cols = [cols8[:, 2:3], cols8[:, 3:4]]
    a8_cols = [cols8[:, 4:5], cols8[:, 5:6]]
    for ch in range(2):
        nc.vector.tensor_mul(a_cols[ch], bcols[:, ch, 4:5], bc[:, ch:ch + 1])
        nc.vector.tensor_mul(cols8[:, 6 + ch:7 + ch], bc[:, 2 + ch:3 + ch],
                             a_cols[ch])
        nc.vector.tensor_sub(b_cols[ch], bcols[:, ch, 5:6],
                             cols8[:, 6 + ch:7 + ch])
        nc.vector.tensor_scalar_mul(a8_cols[ch], a_cols[ch], WS)
    warm(8)
    # a zero column that data-depends on the dance: used to PIN late ACT
    # ops (wo8) behind the stats/dance — without a dep the scheduler
    # hoists them into the ACT stats stream where their wstg wait stalls it.
    zcol = small.tile([P, 1], F32, name="zcol")
    nc.vector.tensor_scalar_mul(zcol, a_cols[0], 0.0)
    # HAM filler through the dance: the dep-free warm matmuls all get
    # scheduled during the stats phase, so the PE goes quiet for the
    # ~17-22us dance window and HAM halves the clock right as the
    # projections start. These reads of the (just-landed) weight pack are
    # naturally pinned to that window.
    for i in range(28):
        nc.tensor.matmul(
            wu_ps[:, 0:C], lhsT=wu_w, rhs=wstg[:, i % 4, i % 2, :],
            start=True, stop=True,
        )
    # ...and a second filler block pinned to the END of the dance (wz
    # depends on cols8) so the density holds right up to the first
    # projection matmul and HAM never drops to K=4 there.
    wz = stage.tile([P, 2 * P], BF16, name="wz", tag="wz")
    nc.vector.tensor_scalar_mul(wz, wstg[:, 0, 0, 0:2 * P], a8_cols[0])
    for _ in range(10):
        nc.tensor.matmul(
            wu_ps[:, 0:2 * P], lhsT=wu_w, rhs=wz, start=True, stop=True
        )

    # scale wq/wk/wv rows by 8*a (per input channel) straight from the f32
    # staging into fp8 tiles. The x8 lift keeps the fp8 entries
    # (~N(0, a/16)) out of e4m3 subnormal range; the PSUM->SBUF casts
    # divide it back out.
    w_s = {}
    for wi, wname in ((0, "wqt"), (1, "wkt"), (2, "wvt")):
        w_s[wname] = const.tile([P, 2, C], FP8, name=f"{wname}_s")
    # wk first: it gates the longest projection stream (all on DVE — the
    # Pool engine's ucode tensor ops force library swaps that stall the
    # whole core).
    for wname, wi in (("wkt", 1), ("wqt", 0), ("wvt", 2)):
        for ci in range(2):
            nc.vector.tensor_scalar_mul(
                w_s[wname][:, ci, :], wstg[:, wi, ci, :], a8_cols[ci]
            )
    wo8 = const.tile([P, 2, C], FP8)

    # ---- projections (all DoubleRow fp8, contraction over full C=256) ----
    # K^T [C, N] fp8: psum[co,nb] = sum_ci wkt8[ci,co].T @ x8[ci, nb] (x8)
    # and q,k = psum/8 + bias. Casts alternate ACT / DVE (ACT is idle until
    # the attention loop's exp stream starts). The first K matmul pair is
    # issued the moment wkt is scaled; the tiny bias matvecs then ride the
    # PE behind it so their serial chain overlaps the projection stream.
    k_sb = const.tile([P, 2, N], FP8)
    q_sb = const.tile([P, 2, NQ], FP8)
    IWS = 1.0 / WS
    proj_ps = [(ps, "mm"), (ps_acc, "acc"), (ps, "mm"), (ps_o, "po"),
               (ps_acc, "acc")]
    _pi = [0]

    def proj_tile():
        pool, tag = proj_ps[_pi[0] % len(proj_ps)]
        _pi[0] += 1
        return pool.tile([P, QB], F32, name="pp", tag=tag)

    _ci = [0]

    def proj_cast(out, in_, bias):
        # only ACT/DVE can read PSUM; the 5-bank psum ring above keeps the
        # MM stream ~2.5 tiles ahead so the 2-engine cast pace (~325ns/tile)
        # doesn't stall the PE.
        i = _ci[0] % 2
        _ci[0] += 1
        if i == 0:
            nc.scalar.activation(out, in_, Act.Identity, bias=bias, scale=IWS)
        else:
            nc.vector.tensor_scalar(
                out=out, in0=in_, scalar1=IWS, scalar2=bias, op0=Alu.mult,
                op1=Alu.add,
            )

    # first K pair (block 0) — only these two proj psums may be in flight
    # before the matvec chain (deeper would deadlock through be_*'s ring
    # reuse of the same PSUM tags).
    pk0 = [proj_tile() for _ in range(2)]
    for co in range(2):
        nc.tensor.matmul(
            pk0[co], lhsT=w_s["wkt"][:, :, co * P:(co + 1) * P],
            rhs=x8[:, :, 0:QB], start=True, stop=True, perf_mode=DR,
        )

    # projection bias columns: be = W b + bias (bf16 matvecs off the staged
    # weights; psum ring across three pools so the chain pipelines)
    b_bf = small.tile([P, 2], BF16, name="b_bf")
    for ch in range(2):
        nc.vector.tensor_copy(out=b_bf[:, ch:ch + 1], in_=b_cols[ch])
    _mvi = [0]
    mv_pools = [(ps_o, "po"), (ps_acc, "acc"), (ps, "mm")]

    def matvec_bias(wi, rhs_cols, bias_idx, out_name, out_dt=F32):
        outs = []
        for co in range(2):
            pool, tag = mv_pools[_mvi[0] % 3]
            _mvi[0] += 1
            pe = pool.tile([P, 1], F32, name="pe_mv", tag=tag)
            for ci in range(2):
                nc.tensor.matmul(
                    pe, lhsT=wstg[:, wi, ci, co * P:(co + 1) * P],
                    rhs=rhs_cols[ci], start=(ci == 0), stop=(ci == 1),
                )
            t = small.tile([P, 1], out_dt, name=f"{out_name}_{co}")
            nc.scalar.activation(
                t, pe, Act.Identity, bias=bcols[:, co, bias_idx:bias_idx + 1],
                scale=1.0,
            )
            outs.append(t)
        return outs

    def warm_wz(n):
        # HAM filler usable inside the matvec chain: wz is ready by then,
        # and these soak up the PE's ~300-500ns waits on the ACT bias reads
        # so the activity monitor doesn't halve the clock mid-chain.
        for _ in range(n):
            nc.tensor.matmul(
                wu_ps[:, 0:2 * P], lhsT=wu_w, rhs=wz, start=True, stop=True
            )

    bcol_list = [b_bf[:, 0:1], b_bf[:, 1:2]]
    be_q = matvec_bias(0, bcol_list, 0, "be_q")
    warm_wz(3)
    be_k = matvec_bias(1, bcol_list, 1, "be_k")
    warm_wz(3)
    vbv = matvec_bias(2, bcol_list, 2, "vbv", out_dt=BF16)
    warm_wz(3)
    bo_eff = matvec_bias(3, vbv, 3, "bo_eff")
    warm_wz(3)
    # dummy exp: pulls the exp table load into the projection phase. The
    # scale AP is a fake data dependency on bo_eff — without it the tile
    # scheduler hoists this exp into the stats phase and the sqrt/exp
    # table sets ping-pong (4 extra 1.5us table loads).
    nc.scalar.activation(warm11, one11, Act.Exp, scale=bo_eff[0][0:1, 0:1])
    for co in range(2):
        proj_cast(k_sb[:, co, 0:QB], pk0[co], be_k[co])
    # wo8 = 8*wo in fp8 (no GroupNorm folding on the out-proj); the zcol
    # bias pins these behind the dance (first consumer is qb0's epilogue,
    # ~20us later).
    for ch in range(2):
        nc.scalar.activation(wo8[:, ch, :], wstg[:, 3, ch, :], Act.Identity,
                             bias=zcol, scale=WS)

    for nb in range(NNB):
        for co in range(2):
            if nb < NQB:
                pq = proj_tile()
                nc.tensor.matmul(
                    pq, lhsT=w_s["wqt"][:, :, co * P:(co + 1) * P],
                    rhs=x8[:, :, nb * QB:(nb + 1) * QB],
                    start=True, stop=True, perf_mode=DR,
                )
                proj_cast(q_sb[:, co, nb * QB:(nb + 1) * QB], pq, be_q[co])
            if nb >= 1:
                pk = proj_tile()
                nc.tensor.matmul(
                    pk, lhsT=w_s["wkt"][:, :, co * P:(co + 1) * P],
                    rhs=x8[:, :, nb * QB:(nb + 1) * QB],
                    start=True, stop=True, perf_mode=DR,
                )
                proj_cast(k_sb[:, co, nb * QB:(nb + 1) * QB], pk, be_k[co])

    # V [N, C] fp8 (bias folded into bo_eff): psum[nt] = x8_tile.T @ wvt8
    v_sb = const.tile([P, NKT, C], FP8)
    v_flat = v_sb.rearrange("p k c -> p (k c)")
    zero_col = const.tile([P, 1], F32)
    nc.vector.memset(zero_col, 0.0)
    for nt in range(0, NKT, 2):
        pv = proj_tile()
        for n2 in range(2):
            nc.tensor.matmul(
                pv[:, n2 * C:(n2 + 1) * C],
                lhsT=x8[:, :, (nt + n2) * P:(nt + n2 + 1) * P],
                rhs=w_s["wvt"][:, :, :],
                start=True, stop=True, perf_mode=DR,
            )
        proj_cast(v_flat[:, nt * C:(nt + 2) * C], pv, zero_col)

    # ---- attention, per query block; DoubleRow over key-tile pairs with a
    # batched exp (one ACT call per pair reading a 2-bank PSUM tile) and the
    # denominator as a DoubleRow ones-matmul on the PE. The softmax division
    # is commuted through the out-projection: out = (wo8 @ PV) * (1/(8*den))
    # + bo_eff + x, deferred one qb so nothing waits on the reciprocal.
    def epilogue_a(qb, dps, aps, at_sb, p_sb_dbg=None, last=False):
        # casts first: they release the PV accumulator banks immediately.
        # 1/8 keeps the heavy-tailed PV numerator inside fp8's +-240.
        # On the tail-exposed final block the ch1 cast runs on ACT (its exp
        # stream is finished) in parallel with DVE's ch0.
        nc.vector.tensor_scalar_mul(at_sb[:, 0, :], aps[0], 1.0 / 8.0)
        if last:
            nc.scalar.activation(at_sb[:, 1, :], aps[1], Act.Identity,
                                 scale=1.0 / 8.0)
        else:
            nc.vector.tensor_scalar_mul(at_sb[:, 1, :], aps[1], 1.0 / 8.0)
        den_r = work.tile([1, QB], F32, name="den_r", tag="den_r")
        nc.vector.reciprocal_approx_fast(out=den_r, in_=dps)
        den_b = work.tile([P, QB], F32, name="den_b", tag="den_b", bufs=2)
        nc.gpsimd.partition_broadcast(den_b, den_r)
        if DEBUG:
            nc.sync.dma_start(
                out=d["dbg_denr"][:, qb * QB:(qb + 1) * QB], in_=den_r
            )
            nc.sync.dma_start(
                out=d["dbg_at"][:, qb * 2 * QB:(qb + 1) * 2 * QB],
                in_=at_sb.rearrange("p h n -> p (h n)"),
            )
            if qb == 0:
                nc.sync.dma_start(
                    out=d["dbg_p"][:, :], in_=p_sb_dbg.rearrange("p k n -> p (k n)")
                )
        return den_b

    def epilogue_co(qb, at_sb, den_b, co, po_pool=None, po_tag="po",
                    dma_split=False):
        po_pool = po_pool or ps_o
        po = po_pool.tile([P, QB], F32, name="po", tag=po_tag)
        nc.tensor.matmul(
            po, lhsT=wo8[:, :, co * P:(co + 1) * P], rhs=at_sb,
            start=True, stop=True, perf_mode=DR,
        )
        t1 = work.tile([P, QB], F32, name="t1", tag="t1")
        nc.vector.tensor_mul(t1, po, den_b)
        res = work.tile([P, QB], F32, name="res", tag="res", bufs=4)
        nc.vector.scalar_tensor_tensor(
            out=res, in0=t1, scalar=bo_eff[co],
            in1=x_sb[:, co, qb * QB:(qb + 1) * QB], op0=Alu.add, op1=Alu.add,
        )
        if dma_split:
            # tail-exposed final block: halve the transfer across both
            # HWDGE queues so the last bytes land ~1.5us earlier.
            HB = QB // 2
            for h, qeng in ((0, nc.sync), (1, nc.scalar)):
                qeng.dma_start(
                    out=out_d[co * P:(co + 1) * P,
                              qb * QB + h * HB:qb * QB + (h + 1) * HB],
                    in_=res[:, h * HB:(h + 1) * HB],
                )
        else:
            nc.sync.dma_start(
                out=out_d[co * P:(co + 1) * P, qb * QB:(qb + 1) * QB], in_=res
            )

    if DEBUG:
        nc.sync.dma_start(out=d["dbg_k"][:, :], in_=k_sb.rearrange("p h n -> p (h n)"))
        nc.sync.dma_start(out=d["dbg_q"][:, :], in_=q_sb.rearrange("p h n -> p (h n)"))
        nc.sync.dma_start(out=d["dbg_v"][:, :], in_=v_flat)

    pending = None
    pend_den = None
    for qb in range(NQB):
        p_sb = pblk.tile([P, NKT, QB], FP8, name="p_sb")
        dps = ps_d.tile([1, QB], F32, name="dps")
        aps = [
            ps_acc.tile([P, QB], F32, name="aps", tag="acc") for _ in range(2)
        ]
        at_sb = work.tile([P, 2, QB], FP8, name="at_sb", tag="at_sb", bufs=2)
        for j in range(NKP + 2):
            if j == 1 and pending is not None:
                pend_den = epilogue_a(*pending)
            if j == 2 and pending is not None:
                epilogue_co(pending[0], pending[3], pend_den, 0)
            if j == 3 and pending is not None:
                epilogue_co(pending[0], pending[3], pend_den, 1)
                pending = None
            if j < NKP:
                sps2 = ps.tile([P, 2 * QB], F32, name="sps2", tag="mm")
                for h2 in range(2):
                    kt = 2 * j + h2
                    nc.tensor.matmul(
                        sps2[:, h2 * QB:(h2 + 1) * QB],
                        lhsT=k_sb[:, :, kt * P:(kt + 1) * P],
                        rhs=q_sb[:, :, qb * QB:(qb + 1) * QB],
                        start=True, stop=True, perf_mode=DR,
                    )
                nc.scalar.activation(
                    p_sb[:, 2 * j:2 * j + 2, :], sps2, Act.Exp,
                    bias=esh_col, scale=SCALE,
                )
            if j >= 2:
                pj = j - 2
                pair = p_sb[:, 2 * pj:2 * pj + 2, :]
                for ch in range(2):
                    nc.tensor.matmul(
                        aps[ch],
                        lhsT=v_sb[:, 2 * pj:2 * pj + 2, ch * P:(ch + 1) * P],
                        rhs=pair,
                        start=(pj == 0), stop=(pj == NKP - 1),
                        perf_mode=DR, skip_group_check=True,
                    )
                nc.tensor.matmul(
                    dps, lhsT=ones8, rhs=pair,
                    start=(pj == 0), stop=(pj == NKP - 1),
                    perf_mode=DR, skip_group_check=True,
                )
        pending = (qb, dps, aps, at_sb, p_sb)
    pend_den = epilogue_a(*pending, last=True)
    epilogue_co(pending[0], pending[3], pend_den, 0, po_pool=ps, po_tag="mm",
                dma_split=True)
    epilogue_co(pending[0], pending[3], pend_den, 1, po_pool=ps, po_tag="mm",
                dma_split=True)

    for pool in (ps_o, ps_d, ps_acc, ps, work, pblk, small, stage, const):
        pool.release()


def build_program():
    global _NC
    if _NC is not None:
        return _NC
    nc = bacc.Bacc("TRN2", target_bir_lowering=False, debug=False,
                   num_devices=NCORES)
    d = {
        "x": nc.dram_tensor("x", [C, N], BF16, kind="ExternalInput"),
        "x8": nc.dram_tensor("x8", [C, N], FP8, kind="ExternalInput"),
        "wpack": nc.dram_tensor("wpack", [4 * C, C], BF16, kind="ExternalInput"),
        "bpack": nc.dram_tensor("bpack", [C, 6], F32, kind="ExternalInput"),
        "m1": nc.dram_tensor("m1", [P, G // 2], F32, kind="ExternalInput"),
        "m2": nc.dram_tensor("m2", [G // 2, P], F32, kind="ExternalInput"),
        "out": nc.dram_tensor("out", [C, NQ], F32, kind="ExternalOutput"),
    }
    if DEBUG:
        d.update({
            "dbg_k": nc.dram_tensor("dbg_k", [P, 2 * N], FP8, kind="ExternalOutput"),
            "dbg_q": nc.dram_tensor("dbg_q", [P, 2 * NQ], FP8, kind="ExternalOutput"),
            "dbg_v": nc.dram_tensor("dbg_v", [P, NKT * C], FP8, kind="ExternalOutput"),
            "dbg_p": nc.dram_tensor("dbg_p", [P, NKT * QB], FP8, kind="ExternalOutput"),
            "dbg_denr": nc.dram_tensor("dbg_denr", [1, NQ], F32, kind="ExternalOutput"),
            "dbg_at": nc.dram_tensor("dbg_at", [P, NQB * 2 * QB], FP8, kind="ExternalOutput"),
        })
    with tile.TileContext(nc) as tc:
        _body(tc, d)
    nc.compile()
    _NC = nc
    return nc


def make_in_maps(x, gamma, beta, wq, bq, wk, bk, wv, bv, wo, bo):
    import ml_dtypes

    f32c = lambda a: np.ascontiguousarray(np.asarray(a, dtype=np.float32))
    x = f32c(x)
    wpack = np.concatenate(
        [f32c(np.asarray(w, np.float32).T) for w in (wq, wk, wv, wo)], axis=0
    )
    bpack = np.stack(
        [f32c(v).reshape(C) for v in (bq, bk, bv, bo, gamma, beta)], axis=1
    )
    m1 = np.zeros((P, G // 2), np.float32)
    for g in range(G // 2):
        m1[8 * g:8 * g + 8, g] = 1.0
    base = {
        "wpack": np.ascontiguousarray(wpack.astype(ml_dtypes.bfloat16)),
        "bpack": np.ascontiguousarray(bpack),
        "m1": m1,
        "m2": np.ascontiguousarray(m1.T),
    }
    import ml_dtypes

    in_maps = []
    for core in range(NCORES):
        b, h = divmod(core, 2)
        xb = x[b].reshape(C, N)
        if h:
            xb = np.concatenate([xb[:, NQ:], xb[:, :NQ]], axis=1)
        in_maps.append({
            **base,
            "x": np.ascontiguousarray(xb.astype(ml_dtypes.bfloat16)),
            "x8": np.ascontiguousarray(xb.astype(ml_dtypes.float8_e4m3)),
        })
    return in_maps


def kernel(x, gamma, beta, wq, bq, wk, bk, wv, bv, wo, bo):
    global LAST_RESULTS
    from concourse.bass_utils import run_bass_kernel_spmd

    nc = build_program()
    in_maps = make_in_maps(x, gamma, beta, wq, bq, wk, bk, wv, bv, wo, bo)
    res = run_bass_kernel_spmd(nc, in_maps, core_ids=list(range(NCORES)))
    LAST_RESULTS = res
    out = np.empty((B, C, N), np.float32)
    for core in range(NCORES):
        b, h = divmod(core, 2)
        out[b][:, h * NQ:(h + 1) * NQ] = res.results[core]["out"]
    return out.reshape(B, C, H, W)



# revision 45
# speedup vs baseline: 1.2501x; 1.0326x over previous
"""AttnBlock (GroupNorm + single-head self-attention + residual) on 8 TRN2 cores.

Sharding: data-parallel over (batch b, query-half h) -> 8 shards. Each core
receives the full [C, N] image of its batch (columns rolled so that its own
query half always occupies columns 0:NQ), computes GroupNorm stats + K/V over
the whole image, Q over its half, and a flash-style attention in which scores
are produced directly transposed (S^T = K^T.T @ Q^T tiles) so softmax
normalization is done via a ones-vector matmul and no PE transposes of P are
needed.

All large matmuls (projections, S^T, PV, denominator, out-proj) run as fp8e4
DoubleRow matmuls: lhsT [128, 2, M] / rhs [128, 2, N] contract 256 deep in a
single instruction at ~2x bf16 FLOP rate. Weights are scaled x8 before the
fp8 cast (their entries are ~N(0, 1/16) and would hit e4m3 subnormals); the
scale is removed in the PSUM->SBUF cast. The softmax exp is shifted by -2
(exp(s/16 - 2)) so P fits e4m3's +-240 range; numerator and denominator share
the shift so the ratio is unchanged. exp runs on ACT in 2-key-tile batches
([128,1024] over a 2-bank PSUM tile) to amortize the per-call overhead, and
the denominator is a DoubleRow ones-matmul on the PE (accumulated in a
[1,512] PSUM bank), keeping the DVE free for casts and the epilogue.
"""

import os
import sys

import numpy as np

for _p in ("/opt/trn_rl_repo", "/root/.axon_site/_ro/trn_rl_repo"):
    if os.path.isdir(_p) and _p not in sys.path:
        sys.path.insert(0, _p)

import concourse.bass as bass  # noqa: E402
import concourse.tile as tile  # noqa: E402
from concourse import bacc, mybir  # noqa: E402
from concourse.masks import make_identity  # noqa: E402

# The agent image's antenv lacks axon_hooks; if BASS_TRACE is set in the
# environment, run_bass_kernel_spmd would crash importing it. Provide a stub
# (profiling degrades gracefully to "hook isn't registered").
try:
    import antenv.axon_hooks  # noqa: F401
except ImportError:
    import types as _types

    _m = _types.ModuleType("antenv.axon_hooks")
    _h = [None]
    _m.set_axon_ntff_profile_hook = lambda h: _h.__setitem__(0, h)
    _m.get_axon_ntff_profile_hook = lambda: _h[0]
    sys.modules["antenv.axon_hooks"] = _m

B, C, H, W = 4, 256, 64, 64
N = H * W  # 4096 pixels
NQ = N // 2  # 2048 queries per core
G = 32  # groups
CPG = C // G  # 8 channels per group
EPS = 1e-5
NCORES = 8
SCALE = float(C) ** -0.5  # 0.0625
ESHIFT = -3.0  # exp(s*SCALE + ESHIFT): data max logit 7.95 -> P <= ~141 < 240
WS = 8.0  # weight fp8 pre-scale (entries ~N(0,1/16) need lifting)

F32 = mybir.dt.float32
BF16 = mybir.dt.bfloat16
FP8 = mybir.dt.float8e4

QB = 512  # query block (free dim of S^T / PV matmuls)
NQB = NQ // QB  # 4 query blocks
NKT = N // 128  # 32 key tiles
NKP = NKT // 2  # 16 key-tile pairs (DoubleRow granularity)
NNB = N // QB  # 8 pixel blocks for K/V projections
P = 128

DEBUG = bool(int(os.environ.get("KDEBUG", "0")))

Act = mybir.ActivationFunctionType
Alu = mybir.AluOpType
Axis = mybir.AxisListType
DR = mybir.MatmulPerfMode.DoubleRow

_NC = None
LAST_RESULTS = None


def _body(tc, d):
    nc = tc.nc
    x_d = d["x"]
    out_d = d["out"]

    const = tc.alloc_tile_pool(name="const", bufs=1)
    stage = tc.alloc_tile_pool(name="stage", bufs=2)
    small = tc.alloc_tile_pool(name="small", bufs=1)
    pblk = tc.alloc_tile_pool(name="pblk", bufs=2)
    work = tc.alloc_tile_pool(name="work", bufs=2)
    # PSUM budget (8 banks): S-pair tiles 2x[P,1024] = 4, aps 2x[P,512] = 2,
    # dps [1,512] = 1, po [P,512] = 1.
    ps = tc.alloc_tile_pool(name="ps", bufs=2, space="PSUM")
    ps_acc = tc.alloc_tile_pool(name="ps_acc", bufs=2, space="PSUM")
    ps_d = tc.alloc_tile_pool(name="ps_d", bufs=1, space="PSUM")
    ps_o = tc.alloc_tile_pool(name="ps_o", bufs=1, space="PSUM")

    # ---- x in SBUF first: bf16 [128, 2(ch), 4096] (residual + stats) and
    # fp8 [128, 2, 4096] (matmul operand), both cast on host; chunked so
    # bn_stats overlaps the transfer ----
    x_sb = const.tile([P, 2, N], BF16)
    x8 = const.tile([P, 2, N], FP8)
    x_src = x_d.ap().rearrange("(h p) n -> p h n", p=P)
    x8_src = d["x8"].ap().rearrange("(h p) n -> p h n", p=P)

    # PE warm-up: keep the HAM activity monitor busy during the DMA/stats
    # window so projections and attention run at full clock from the start.
    # Memsets go FIRST so the warm matmuls aren't queued behind the bn_stats
    # chain in DVE program order (PE would idle for the whole DMA phase).
    wu_w = const.tile([P, P], BF16)
    nc.vector.memset(wu_w, 0.0)
    wu_x = const.tile([P, 2 * P], BF16)
    nc.vector.memset(wu_x, 0.0)
    wu_ps = ps.tile([P, QB], F32, name="wu_ps", tag="mm")

    def warm(n):
        for _ in range(n):
            nc.tensor.matmul(
                wu_ps[:, 0:2 * P], lhsT=wu_w, rhs=wu_x, start=True, stop=True
            )

    warm(2)

    # ---- head DMA: x8 (1MB, feeds stats AND all matmuls) goes FIRST in
    # 1024-col per-channel chunks: ch1 leads on both queues so the ACT
    # stats units and the DVE's first units have data earliest. ACT's own
    # HWDGE queue carries NO early DMA (each issue costs ~0.7us of ACT
    # sequencer time and starved the act-table loads in an earlier rev):
    # only the weight pack + one x_sb chunk ride scalar, the rest goes
    # sync + gpsimd (SWDGE).
    # Both-channel column chunks (1024B+ contiguous per partition transfers
    # ~90B/ns/queue; channel-split halves that). A small 512-col first
    # chunk lands ~1.5us earlier so the stats stream starts sooner.
    x8_chunks = [
        (nc.sync, 0, 1), (nc.scalar, 1, 2), (nc.gpsimd, 4, 6),
        (nc.sync, 2, 4), (nc.gpsimd, 6, 8),
    ]
    for q, u0, u1 in x8_chunks:
        sl = (slice(None), slice(None), slice(u0 * QB, u1 * QB))
        q.dma_start(out=x8[sl], in_=x8_src[sl])
    # all four weight matrices (bf16) in one DMA on the scalar queue (its
    # only early issue; needed at ~17us for the w_s scaling)
    wstg = stage.tile([P, 4, 2, C], BF16, name="wstg", tag="wstg")
    nc.scalar.dma_start(
        out=wstg, in_=d["wpack"].ap().rearrange("(w h p) co -> p w h co", p=P, h=2)
    )
    m1_sb = const.tile([P, G // 2], F32)
    nc.sync.dma_start(out=m1_sb, in_=d["m1"][:, :])
    m2_sb = const.tile([G // 2, P], F32)
    nc.sync.dma_start(out=m2_sb, in_=d["m2"][:, :])
    # packed biases/affine, pre-permuted on host to [128, 12] with column
    # j = 2*s + co for s in (bq,bk,bv,bo,gamma,beta)
    bcols = const.tile([P, 12], F32)
    nc.sync.dma_start(out=bcols, in_=d["bpack"][:, :])
    xsbq = [nc.sync, nc.sync, nc.gpsimd, nc.gpsimd]
    for c in range(4):
        sl = (slice(None), slice(None), slice(c * 2 * QB, (c + 1) * 2 * QB))
        xsbq[c].dma_start(out=x_sb[sl], in_=x_src[sl])

    # GroupNorm stats from the fp8 x (self-consistent: the projections
    # consume the same fp8 values; adds ~0.1% to the error budget). Split
    # across DVE (bn_stats: ch0 + ch1 units 0,1,2,3,7) and ACT (Square +
    # Identity passes with free-axis accumulate: ch1 units 4,5,6). Every
    # ACT table set contains identity+copy+square, so the sqrt set
    # (preloaded by the dummy sqrt below) serves the stats AND the dance;
    # the only other table load is the exp set, pinned into the projection
    # phase by the dummy exp's fake bo_eff dependency.
    eps16 = const.tile([G // 2, 1], F32)
    nc.vector.memset(eps16, EPS)
    one11 = const.tile([1, 1], F32)
    nc.vector.memset(one11, 1.0)
    warm11 = small.tile([1, 1], F32)
    nc.scalar.activation(warm11, one11, Act.Sqrt, scale=1.0)

    bn_st0 = small.tile([P, NNB, 6], F32, name="bn_st0")
    bn_st1b = small.tile([P, 5, 6], F32, name="bn_st1b")
    ACT_UNITS = (4, 5, 6)
    sq5 = small.tile([P, len(ACT_UNITS)], F32, name="sq5")
    sm5 = small.tile([P, len(ACT_UNITS)], F32, name="sm5")
    scr_bf = stage.tile([P, QB], BF16, name="scr_bf", tag="scr")
    for i, u in enumerate(ACT_UNITS):
        cols = slice(u * QB, (u + 1) * QB)
        nc.scalar.activation(
            scr_bf, x8[:, 1, cols], Act.Square, accum_out=sq5[:, i:i + 1]
        )
        nc.scalar.activation(
            scr_bf, x8[:, 1, cols], Act.Identity, accum_out=sm5[:, i:i + 1]
        )
    # DVE bn_stats in data-arrival order: small first chunk (u0), u1,
    # then gpsimd chunk (ch0 u4,u5), sync chunk (u2,u3), gpsimd tail.
    dve_units = [(1, 0, 0), (0, 0, None), (1, 1, 1), (0, 1, None),
                 (0, 4, None), (0, 5, None),
                 (1, 2, 2), (1, 3, 3), (0, 2, None), (0, 3, None),
                 (1, 7, 4), (0, 6, None), (0, 7, None)]
    for i, (ch, u, b1) in enumerate(dve_units):
        dst = bn_st0[:, u, :] if ch == 0 else bn_st1b[:, b1, :]
        nc.vector.bn_stats(out=dst, in_=x8[:, ch, u * QB:(u + 1) * QB])
        if i % 2 == 1:
            warm(6)

    # ---- constants ----
    # padded to 16B so the DoubleRow k-tile stride meets walrus' step%16==0.
    # Value 1.0 balances the at_sb 1/8 down-scale and the x8 wo lift:
    # po*den_r = (8/8)*wo@at_un / den = wo@at_un/den.
    ones8_pad = const.tile([P, 2, 16], FP8)
    nc.vector.memset(ones8_pad, 1.0)
    ones8 = ones8_pad[:, :, 0:1]
    esh_col = const.tile([P, 1], F32)
    nc.vector.memset(esh_col, ESHIFT)

    # ---- GroupNorm dance: per-channel (mean, E[x^2]) -> group reduce via
    # mask matmul -> rstd -> broadcast back via mask matmul -> a/b columns.
    # ch0 comes straight from bn_aggr; ch1 blends the 5 DVE units (2560
    # cols, weight 0.625) with the ACT accumulator sums (1536 cols).
    mvex = small.tile([P, 2, 2], F32, name="mvex")  # (mean, ex2) per ch-half
    mv0 = small.tile([P, 2], F32, name="mv0")
    nc.vector.bn_aggr(out=mv0, in_=bn_st0)
    mv1b = small.tile([P, 2], F32, name="mv1b")
    nc.vector.bn_aggr(out=mv1b, in_=bn_st1b)
    scol = small.tile([P, 2], F32, name="scol")
    nc.vector.tensor_reduce(
        out=scol[:, 0:1], in_=sm5, axis=Axis.X, op=Alu.add
    )
    nc.vector.tensor_reduce(
        out=scol[:, 1:2], in_=sq5, axis=Axis.X, op=Alu.add
    )
    tcol = small.tile([P, 6], F32, name="tcol")
    # ch0: mean, ex2 = var + mean^2
    nc.vector.tensor_copy(out=mvex[:, 0, 0:1], in_=mv0[:, 0:1])
    nc.vector.tensor_mul(tcol[:, 0:1], mv0[:, 0:1], mv0[:, 0:1])
    nc.vector.tensor_add(mvex[:, 0, 1:2], tcol[:, 0:1], mv0[:, 1:2])
    # ch1: mean = FB*mean_b + sum_id/4096
    FB = 2560.0 / 4096.0
    nc.vector.tensor_scalar_mul(tcol[:, 1:2], scol[:, 0:1], 1.0 / 4096.0)
    nc.vector.scalar_tensor_tensor(
        out=mvex[:, 1, 0:1], in0=mv1b[:, 0:1], scalar=FB, in1=tcol[:, 1:2],
        op0=Alu.mult, op1=Alu.add,
    )
    # ch1: ex2 = FB*(var_b + mean_b^2) + sum_sq/4096
    nc.vector.tensor_mul(tcol[:, 2:3], mv1b[:, 0:1], mv1b[:, 0:1])
    nc.vector.tensor_add(tcol[:, 3:4], mv1b[:, 1:2], tcol[:, 2:3])
    nc.vector.tensor_scalar_mul(tcol[:, 4:5], scol[:, 1:2], 1.0 / 4096.0)
    nc.vector.scalar_tensor_tensor(
        out=mvex[:, 1, 1:2], in0=tcol[:, 3:4], scalar=FB, in1=tcol[:, 4:5],
        op0=Alu.mult, op1=Alu.add,
    )
    # group sums over partitions: [16 groups, (mean0, ex20, mean1, ex21)]
    gsum = ps_o.tile([G // 2, 4], F32, name="gsum", tag="po")
    nc.tensor.matmul(
        gsum, lhsT=m1_sb, rhs=mvex.rearrange("p c k -> p (c k)"),
        start=True, stop=True,
    )
    warm(8)
    vals = small.tile([G // 2, 4], F32, name="vals")  # (rstd0, rstd1, m0, m1)
    gtmp = small.tile([G // 2, 4], F32, name="gtmp")
    gview = gsum.rearrange("g (c k) -> g c k", k=2)
    nc.vector.tensor_scalar_mul(
        vals.rearrange("g (c k) -> g c k", k=2)[:, 1, :], gview[:, :, 0],
        1.0 / CPG,
    )
    nc.vector.tensor_scalar_mul(gtmp[:, 0:2], gview[:, :, 1], 1.0 / CPG)
    nc.vector.tensor_mul(gtmp[:, 2:4], vals[:, 2:4], vals[:, 2:4])
    nc.vector.tensor_sub(gtmp[:, 0:2], gtmp[:, 0:2], gtmp[:, 2:4])
    # rstd = 1/sqrt(var + eps) (sqrt table preloaded; recip on DVE)
    nc.scalar.activation(gtmp[:, 2:4], gtmp[:, 0:2], Act.Sqrt, bias=eps16)
    nc.vector.reciprocal(vals[:, 0:2], gtmp[:, 2:4])
    # broadcast back to channels: [128, (rstd0, rstd1, m0, m1)]
    bc = ps_acc.tile([P, 4], F32, name="bc", tag="acc")
    nc.tensor.matmul(bc, lhsT=m2_sb, rhs=vals, start=True, stop=True)
    warm(8)
    # a = gamma * rstd; b = beta - mean * a; a8 = 8a (per channel cols)
    cols8 = small.tile([P, 8], F32, name="cols8")
    a_cols = [cols8[:, 0:1], cols8[:, 1:2]]
    b_cols = [cols8[:, 2:3], cols8[:, 3:4]]
    a8_cols = [cols8[:, 4:5], cols8[:, 5:6]]
    for ch in range(2):
        nc.vector.tensor_mul(a_cols[ch], bcols[:, 8 + ch:9 + ch],
                             bc[:, ch:ch + 1])
        nc.vector.tensor_mul(cols8[:, 6 + ch:7 + ch], bc[:, 2 + ch:3 + ch],
                             a_cols[ch])
        nc.vector.tensor_sub(b_cols[ch], bcols[:, 10 + ch:11 + ch],
                             cols8[:, 6 + ch:7 + ch])
        nc.vector.tensor_scalar_mul(a8_cols[ch], a_cols[ch], WS)
    warm(8)
    # a zero column that data-depends on the dance: used to PIN late ACT
    # ops (wo8) behind the stats/dance — without a dep the scheduler
    # hoists them into the ACT stats stream where their wstg wait stalls it.
    zcol = small.tile([P, 1], F32, name="zcol")
    nc.vector.tensor_scalar_mul(zcol, a_cols[0], 0.0)
    # HAM filler through the dance: the dep-free warm matmuls all get
    # scheduled during the stats phase, so the PE goes quiet for the
    # ~17-22us dance window and HAM halves the clock right as the
    # projections start. These reads of the (just-landed) weight pack are
    # naturally pinned to that window.
    for i in range(28):
        nc.tensor.matmul(
            wu_ps[:, 0:C], lhsT=wu_w, rhs=wstg[:, i % 4, i % 2, :],
            start=True, stop=True,
        )

    # scale wq/wk/wv rows by 8*a (per input channel) straight from the f32
    # staging into fp8 tiles. The x8 lift keeps the fp8 entries
    # (~N(0, a/16)) out of e4m3 subnormal range; the PSUM->SBUF casts
    # divide it back out.
    w_s = {}
    for wi, wname in ((0, "wqt"), (1, "wkt"), (2, "wvt")):
        w_s[wname] = const.tile([P, 2, C], FP8, name=f"{wname}_s")
    # wk first: it gates the longest projection stream (all on DVE — the
    # Pool engine's ucode tensor ops force library swaps that stall the
    # whole core).
    for wname, wi in (("wkt", 1), ("wqt", 0), ("wvt", 2)):
        for ci in range(2):
            nc.vector.tensor_scalar_mul(
                w_s[wname][:, ci, :], wstg[:, wi, ci, :], a8_cols[ci]
            )
    wo8 = const.tile([P, 2, C], FP8)

    # ---- projections (all DoubleRow fp8, contraction over full C=256) ----
    # K^T [C, N] fp8: psum[co,nb] = sum_ci wkt8[ci,co].T @ x8[ci, nb] (x8)
    # and q,k = psum/8 + bias. Casts alternate ACT / DVE (ACT is idle until
    # the attention loop's exp stream starts). The first K matmul pair is
    # issued the moment wkt is scaled; the tiny bias matvecs then ride the
    # PE behind it so their serial chain overlaps the projection stream.
    k_sb = const.tile([P, 2, N], FP8)
    q_sb = const.tile([P, 2, NQ], FP8)
    IWS = 1.0 / WS
    proj_ps = [(ps, "mm"), (ps_acc, "acc"), (ps, "mm"), (ps_o, "po"),
               (ps_acc, "acc")]
    _pi = [0]

    def proj_tile():
        pool, tag = proj_ps[_pi[0] % len(proj_ps)]
        _pi[0] += 1
        return pool.tile([P, QB], F32, name="pp", tag=tag)

    _ci = [0]

    def proj_cast(out, in_, bias):
        # only ACT/DVE can read PSUM; the 5-bank psum ring above keeps the
        # MM stream ~2.5 tiles ahead so the 2-engine cast pace (~325ns/tile)
        # doesn't stall the PE.
        i = _ci[0] % 2
        _ci[0] += 1
        if i == 0:
            nc.scalar.activation(out, in_, Act.Identity, bias=bias, scale=IWS)
        else:
            nc.vector.tensor_scalar(
                out=out, in0=in_, scalar1=IWS, scalar2=bias, op0=Alu.mult,
                op1=Alu.add,
            )

    # first K pair (block 0) — only these two proj psums may be in flight
    # before the matvec chain (deeper would deadlock through be_*'s ring
    # reuse of the same PSUM tags).
    pk0 = [proj_tile() for _ in range(2)]
    for co in range(2):
        nc.tensor.matmul(
            pk0[co], lhsT=w_s["wkt"][:, :, co * P:(co + 1) * P],
            rhs=x8[:, :, 0:QB], start=True, stop=True, perf_mode=DR,
        )

    # projection bias columns: be = W b + bias. All 12 first-level matvecs
    # (wq/wk/wv x co x ci) land in disjoint columns of ONE psum tile, so
    # the PE runs them back-to-back with no ring WARs or ACT round-trips
    # (the old 3-pool ring chain left the PE sparse enough that HAM halved
    # the clock); one strided DVE add applies all six biases at once.
    b_bf = small.tile([P, 2], BF16, name="b_bf")
    for ch in range(2):
        nc.vector.tensor_copy(out=b_bf[:, ch:ch + 1], in_=b_cols[ch])
    mv8 = ps.tile([P, 8], F32, name="mv8", tag="mm")
    for wi in range(3):
        for co in range(2):
            j = 2 * wi + co
            for ci in range(2):
                nc.tensor.matmul(
                    mv8[:, j:j + 1],
                    lhsT=wstg[:, wi, ci, co * P:(co + 1) * P],
                    rhs=b_bf[:, ci:ci + 1], start=(ci == 0), stop=(ci == 1),
                    skip_group_check=True,
                )
    be6 = small.tile([P, 6], F32, name="be6")
    nc.vector.tensor_add(be6, mv8[:, 0:6], bcols[:, 0:6])
    be_q = [be6[:, 0:1], be6[:, 1:2]]
    be_k = [be6[:, 2:3], be6[:, 3:4]]
    v_bf = small.tile([P, 2], BF16, name="v_bf")
    nc.vector.tensor_copy(out=v_bf, in_=be6[:, 4:6])
    # second level: bo_eff = wo @ vbv + bo (4 more dense matmuls)
    for co in range(2):
        for ci in range(2):
            nc.tensor.matmul(
                mv8[:, 6 + co:7 + co],
                lhsT=wstg[:, 3, ci, co * P:(co + 1) * P],
                rhs=v_bf[:, ci:ci + 1], start=(ci == 0), stop=(ci == 1),
                skip_group_check=True,
            )
    bo2 = small.tile([P, 2], F32, name="bo2")
    nc.vector.tensor_add(bo2, mv8[:, 6:8], bcols[:, 6:8])
    bo_eff = [bo2[:, 0:1], bo2[:, 1:2]]
    # dummy exp: pulls the exp table load into the projection phase. The
    # scale AP is a fake data dependency on bo_eff — without it the tile
    # scheduler hoists this exp into the stats phase and the sqrt/exp
    # table sets ping-pong (4 extra 1.5us table loads).
    nc.scalar.activation(warm11, one11, Act.Exp, scale=bo2[0:1, 0:1])
    for co in range(2):
        proj_cast(k_sb[:, co, 0:QB], pk0[co], be_k[co])
    # wo8 = 8*wo in fp8 (no GroupNorm folding on the out-proj); the zcol
    # bias pins these behind the dance (first consumer is qb0's epilogue,
    # ~20us later).
    for ch in range(2):
        nc.scalar.activation(wo8[:, ch, :], wstg[:, 3, ch, :], Act.Identity,
                             bias=zcol, scale=WS)

    for nb in range(NNB):
        for co in range(2):
            if nb < NQB:
                pq = proj_tile()
                nc.tensor.matmul(
                    pq, lhsT=w_s["wqt"][:, :, co * P:(co + 1) * P],
                    rhs=x8[:, :, nb * QB:(nb + 1) * QB],
                    start=True, stop=True, perf_mode=DR,
                )
                proj_cast(q_sb[:, co, nb * QB:(nb + 1) * QB], pq, be_q[co])
            if nb >= 1:
                pk = proj_tile()
                nc.tensor.matmul(
                    pk, lhsT=w_s["wkt"][:, :, co * P:(co + 1) * P],
                    rhs=x8[:, :, nb * QB:(nb + 1) * QB],
                    start=True, stop=True, perf_mode=DR,
                )
                proj_cast(k_sb[:, co, nb * QB:(nb + 1) * QB], pk, be_k[co])

    # V [N, C] fp8 (bias folded into bo_eff): psum[nt] = x8_tile.T @ wvt8
    v_sb = const.tile([P, NKT, C], FP8)
    v_flat = v_sb.rearrange("p k c -> p (k c)")
    zero_col = const.tile([P, 1], F32)
    nc.vector.memset(zero_col, 0.0)
    for nt in range(0, NKT, 2):
        pv = proj_tile()
        for n2 in range(2):
            nc.tensor.matmul(
                pv[:, n2 * C:(n2 + 1) * C],
                lhsT=x8[:, :, (nt + n2) * P:(nt + n2 + 1) * P],
                rhs=w_s["wvt"][:, :, :],
                start=True, stop=True, perf_mode=DR,
            )
        proj_cast(v_flat[:, nt * C:(nt + 2) * C], pv, zero_col)

    # ---- attention, per query block; DoubleRow over key-tile pairs with a
    # batched exp (one ACT call per pair reading a 2-bank PSUM tile) and the
    # denominator as a DoubleRow ones-matmul on the PE. The softmax division
    # is commuted through the out-projection: out = (wo8 @ PV) * (1/(8*den))
    # + bo_eff + x, deferred one qb so nothing waits on the reciprocal.
    def epilogue_a(qb, dps, aps, at_sb, p_sb_dbg=None):
        # casts first: they release the PV accumulator banks immediately.
        # 1/8 keeps the heavy-tailed PV numerator inside fp8's +-240.
        nc.vector.tensor_scalar_mul(at_sb[:, 0, :], aps[0], 1.0 / 8.0)
        nc.vector.tensor_scalar_mul(at_sb[:, 1, :], aps[1], 1.0 / 8.0)
        den_r = work.tile([1, QB], F32, name="den_r", tag="den_r")
        nc.vector.reciprocal_approx_fast(out=den_r, in_=dps)
        den_b = work.tile([P, QB], F32, name="den_b", tag="den_b", bufs=2)
        nc.gpsimd.partition_broadcast(den_b, den_r)
        if DEBUG:
            nc.sync.dma_start(
                out=d["dbg_denr"][:, qb * QB:(qb + 1) * QB], in_=den_r
            )
            nc.sync.dma_start(
                out=d["dbg_at"][:, qb * 2 * QB:(qb + 1) * 2 * QB],
                in_=at_sb.rearrange("p h n -> p (h n)"),
            )
            if qb == 0:
                nc.sync.dma_start(
                    out=d["dbg_p"][:, :], in_=p_sb_dbg.rearrange("p k n -> p (k n)")
                )
        return den_b

    def epilogue_co(qb, at_sb, den_b, co, po_pool=None, po_tag="po",
                    dma_split=False):
        po_pool = po_pool or ps_o
        po = po_pool.tile([P, QB], F32, name="po", tag=po_tag)
        nc.tensor.matmul(
            po, lhsT=wo8[:, :, co * P:(co + 1) * P], rhs=at_sb,
            start=True, stop=True, perf_mode=DR,
        )
        t1 = work.tile([P, QB], F32, name="t1", tag="t1")
        nc.vector.tensor_mul(t1, po, den_b)
        res = work.tile([P, QB], F32, name="res", tag="res", bufs=4)
        nc.vector.scalar_tensor_tensor(
            out=res, in0=t1, scalar=bo_eff[co],
            in1=x_sb[:, co, qb * QB:(qb + 1) * QB], op0=Alu.add, op1=Alu.add,
        )
        if dma_split:
            # tail-exposed final block: halve the transfer across both
            # HWDGE queues so the last bytes land ~1.5us earlier.
            HB = QB // 2
            for h, qeng in ((0, nc.sync), (1, nc.scalar)):
                qeng.dma_start(
                    out=out_d[co * P:(co + 1) * P,
                              qb * QB + h * HB:qb * QB + (h + 1) * HB],
                    in_=res[:, h * HB:(h + 1) * HB],
                )
        else:
            nc.sync.dma_start(
                out=out_d[co * P:(co + 1) * P, qb * QB:(qb + 1) * QB], in_=res
            )

    if DEBUG:
        nc.sync.dma_start(out=d["dbg_k"][:, :], in_=k_sb.rearrange("p h n -> p (h n)"))
        nc.sync.dma_start(out=d["dbg_q"][:, :], in_=q_sb.rearrange("p h n -> p (h n)"))
        nc.sync.dma_start(out=d["dbg_v"][:, :], in_=v_flat)

    pending = None
    pend_den = None
    for qb in range(NQB):
        p_sb = pblk.tile([P, NKT, QB], FP8, name="p_sb")
        dps = ps_d.tile([1, QB], F32, name="dps")
        aps = [
            ps_acc.tile([P, QB], F32, name="aps", tag="acc") for _ in range(2)
        ]
        at_sb = work.tile([P, 2, QB], FP8, name="at_sb", tag="at_sb", bufs=2)
        for j in range(NKP + 2):
            if j == 1 and pending is not None:
                pend_den = epilogue_a(*pending)
            if j == 2 and pending is not None:
                epilogue_co(pending[0], pending[3], pend_den, 0)
            if j == 3 and pending is not None:
                epilogue_co(pending[0], pending[3], pend_den, 1)
                pending = None
            if j < NKP:
                sps2 = ps.tile([P, 2 * QB], F32, name="sps2", tag="mm")
                for h2 in range(2):
                    kt = 2 * j + h2
                    nc.tensor.matmul(
                        sps2[:, h2 * QB:(h2 + 1) * QB],
                        lhsT=k_sb[:, :, kt * P:(kt + 1) * P],
                        rhs=q_sb[:, :, qb * QB:(qb + 1) * QB],
                        start=True, stop=True, perf_mode=DR,
                    )
                nc.scalar.activation(
                    p_sb[:, 2 * j:2 * j + 2, :], sps2, Act.Exp,
                    bias=esh_col, scale=SCALE,
                )
            if j >= 2:
                pj = j - 2
                pair = p_sb[:, 2 * pj:2 * pj + 2, :]
                for ch in range(2):
                    nc.tensor.matmul(
                        aps[ch],
                        lhsT=v_sb[:, 2 * pj:2 * pj + 2, ch * P:(ch + 1) * P],
                        rhs=pair,
                        start=(pj == 0), stop=(pj == NKP - 1),
                        perf_mode=DR, skip_group_check=True,
                    )
                nc.tensor.matmul(
                    dps, lhsT=ones8, rhs=pair,
                    start=(pj == 0), stop=(pj == NKP - 1),
                    perf_mode=DR, skip_group_check=True,
                )
        pending = (qb, dps, aps, at_sb, p_sb)
    pend_den = epilogue_a(*pending)
    epilogue_co(pending[0], pending[3], pend_den, 0, po_pool=ps, po_tag="mm",
                dma_split=True)
    epilogue_co(pending[0], pending[3], pend_den, 1, po_pool=ps, po_tag="mm",
                dma_split=True)

    for pool in (ps_o, ps_d, ps_acc, ps, work, pblk, small, stage, const):
        pool.release()


def build_program():
    global _NC
    if _NC is not None:
        return _NC
    nc = bacc.Bacc("TRN2", target_bir_lowering=False, debug=False,
                   num_devices=NCORES)
    d = {
        "x": nc.dram_tensor("x", [C, N], BF16, kind="ExternalInput"),
        "x8": nc.dram_tensor("x8", [C, N], FP8, kind="ExternalInput"),
        "wpack": nc.dram_tensor("wpack", [4 * C, C], BF16, kind="ExternalInput"),
        "bpack": nc.dram_tensor("bpack", [P, 12], F32, kind="ExternalInput"),
        "m1": nc.dram_tensor("m1", [P, G // 2], F32, kind="ExternalInput"),
        "m2": nc.dram_tensor("m2", [G // 2, P], F32, kind="ExternalInput"),
        "out": nc.dram_tensor("out", [C, NQ], F32, kind="ExternalOutput"),
    }
    if DEBUG:
        d.update({
            "dbg_k": nc.dram_tensor("dbg_k", [P, 2 * N], FP8, kind="ExternalOutput"),
            "dbg_q": nc.dram_tensor("dbg_q", [P, 2 * NQ], FP8, kind="ExternalOutput"),
            "dbg_v": nc.dram_tensor("dbg_v", [P, NKT * C], FP8, kind="ExternalOutput"),
            "dbg_p": nc.dram_tensor("dbg_p", [P, NKT * QB], FP8, kind="ExternalOutput"),
            "dbg_denr": nc.dram_tensor("dbg_denr", [1, NQ], F32, kind="ExternalOutput"),
            "dbg_at": nc.dram_tensor("dbg_at", [P, NQB * 2 * QB], FP8, kind="ExternalOutput"),
        })
    with tile.TileContext(nc) as tc:
        _body(tc, d)
    nc.compile()
    _NC = nc
    return nc


def make_in_maps(x, gamma, beta, wq, bq, wk, bk, wv, bv, wo, bo):
    import ml_dtypes

    f32c = lambda a: np.ascontiguousarray(np.asarray(a, dtype=np.float32))
    x = f32c(x)
    wpack = np.concatenate(
        [f32c(np.asarray(w, np.float32).T) for w in (wq, wk, wv, wo)], axis=0
    )
    bp6 = np.stack(
        [f32c(v).reshape(C) for v in (bq, bk, bv, bo, gamma, beta)], axis=1
    )  # [C, 6]
    bpack = np.empty((P, 12), np.float32)
    for s in range(6):
        for co in range(2):
            bpack[:, 2 * s + co] = bp6[co * P:(co + 1) * P, s]
    m1 = np.zeros((P, G // 2), np.float32)
    for g in range(G // 2):
        m1[8 * g:8 * g + 8, g] = 1.0
    base = {
        "wpack": np.ascontiguousarray(wpack.astype(ml_dtypes.bfloat16)),
        "bpack": np.ascontiguousarray(bpack),
        "m1": m1,
        "m2": np.ascontiguousarray(m1.T),
    }
    import ml_dtypes

    in_maps = []
    for core in range(NCORES):
        b, h = divmod(core, 2)
        xb = x[b].reshape(C, N)
        if h:
            xb = np.concatenate([xb[:, NQ:], xb[:, :NQ]], axis=1)
        in_maps.append({
            **base,
            "x": np.ascontiguousarray(xb.astype(ml_dtypes.bfloat16)),
            "x8": np.ascontiguousarray(xb.astype(ml_dtypes.float8_e4m3)),
        })
    return in_maps


def kernel(x, gamma, beta, wq, bq, wk, bk, wv, bv, wo, bo):
    global LAST_RESULTS
    from concourse.bass_utils import run_bass_kernel_spmd

    nc = build_program()
    in_maps = make_in_maps(x, gamma, beta, wq, bq, wk, bk, wv, bv, wo, bo)
    res = run_bass_kernel_spmd(nc, in_maps, core_ids=list(range(NCORES)))
    LAST_RESULTS = res
    out = np.empty((B, C, N), np.float32)
    for core in range(NCORES):
        b, h = divmod(core, 2)
        out[b][:, h * NQ:(h + 1) * NQ] = res.results[core]["out"]
    return out.reshape(B, C, H, W)



# revision 46
# speedup vs baseline: 1.2608x; 1.0085x over previous
"""AttnBlock (GroupNorm + single-head self-attention + residual) on 8 TRN2 cores.

Sharding: data-parallel over (batch b, query-half h) -> 8 shards. Each core
receives the full [C, N] image of its batch (columns rolled so that its own
query half always occupies columns 0:NQ), computes GroupNorm stats + K/V over
the whole image, Q over its half, and a flash-style attention in which scores
are produced directly transposed (S^T = K^T.T @ Q^T tiles) so softmax
normalization is done via a ones-vector matmul and no PE transposes of P are
needed.

All large matmuls (projections, S^T, PV, denominator, out-proj) run as fp8e4
DoubleRow matmuls: lhsT [128, 2, M] / rhs [128, 2, N] contract 256 deep in a
single instruction at ~2x bf16 FLOP rate. Weights are scaled x8 before the
fp8 cast (their entries are ~N(0, 1/16) and would hit e4m3 subnormals); the
scale is removed in the PSUM->SBUF cast. The softmax exp is shifted by -2
(exp(s/16 - 2)) so P fits e4m3's +-240 range; numerator and denominator share
the shift so the ratio is unchanged. exp runs on ACT in 2-key-tile batches
([128,1024] over a 2-bank PSUM tile) to amortize the per-call overhead, and
the denominator is a DoubleRow ones-matmul on the PE (accumulated in a
[1,512] PSUM bank), keeping the DVE free for casts and the epilogue.
"""

import os
import sys

import numpy as np

for _p in ("/opt/trn_rl_repo", "/root/.axon_site/_ro/trn_rl_repo"):
    if os.path.isdir(_p) and _p not in sys.path:
        sys.path.insert(0, _p)

import concourse.bass as bass  # noqa: E402
import concourse.tile as tile  # noqa: E402
from concourse import bacc, mybir  # noqa: E402
from concourse.masks import make_identity  # noqa: E402

# The agent image's antenv lacks axon_hooks; if BASS_TRACE is set in the
# environment, run_bass_kernel_spmd would crash importing it. Provide a stub
# (profiling degrades gracefully to "hook isn't registered").
try:
    import antenv.axon_hooks  # noqa: F401
except ImportError:
    import types as _types

    _m = _types.ModuleType("antenv.axon_hooks")
    _h = [None]
    _m.set_axon_ntff_profile_hook = lambda h: _h.__setitem__(0, h)
    _m.get_axon_ntff_profile_hook = lambda: _h[0]
    sys.modules["antenv.axon_hooks"] = _m

B, C, H, W = 4, 256, 64, 64
N = H * W  # 4096 pixels
NQ = N // 2  # 2048 queries per core
G = 32  # groups
CPG = C // G  # 8 channels per group
EPS = 1e-5
NCORES = 8
SCALE = float(C) ** -0.5  # 0.0625
ESHIFT = -3.0  # exp(s*SCALE + ESHIFT): data max logit 7.95 -> P <= ~141 < 240
WS = 8.0  # weight fp8 pre-scale (entries ~N(0,1/16) need lifting)

F32 = mybir.dt.float32
BF16 = mybir.dt.bfloat16
FP8 = mybir.dt.float8e4

QB = 512  # query block (free dim of S^T / PV matmuls)
NQB = NQ // QB  # 4 query blocks
NKT = N // 128  # 32 key tiles
NKP = NKT // 2  # 16 key-tile pairs (DoubleRow granularity)
NNB = N // QB  # 8 pixel blocks for K/V projections
P = 128

DEBUG = bool(int(os.environ.get("KDEBUG", "0")))

Act = mybir.ActivationFunctionType
Alu = mybir.AluOpType
Axis = mybir.AxisListType
DR = mybir.MatmulPerfMode.DoubleRow

_NC = None
LAST_RESULTS = None


def _body(tc, d):
    nc = tc.nc
    x_d = d["x"]
    out_d = d["out"]

    const = tc.alloc_tile_pool(name="const", bufs=1)
    stage = tc.alloc_tile_pool(name="stage", bufs=2)
    small = tc.alloc_tile_pool(name="small", bufs=1)
    pblk = tc.alloc_tile_pool(name="pblk", bufs=2)
    work = tc.alloc_tile_pool(name="work", bufs=2)
    # PSUM budget (8 banks): S-pair tiles 2x[P,1024] = 4, aps 2x[P,512] = 2,
    # dps [1,512] = 1, po [P,512] = 1.
    ps = tc.alloc_tile_pool(name="ps", bufs=2, space="PSUM")
    ps_acc = tc.alloc_tile_pool(name="ps_acc", bufs=2, space="PSUM")
    ps_d = tc.alloc_tile_pool(name="ps_d", bufs=1, space="PSUM")
    ps_o = tc.alloc_tile_pool(name="ps_o", bufs=1, space="PSUM")

    # ---- x in SBUF first: bf16 [128, 2(ch), 4096] (residual + stats) and
    # fp8 [128, 2, 4096] (matmul operand), both cast on host; chunked so
    # bn_stats overlaps the transfer ----
    x_sb = const.tile([P, 2, N], BF16)
    x8 = const.tile([P, 2, N], FP8)
    x_src = x_d.ap().rearrange("(h p) n -> p h n", p=P)
    x8_src = d["x8"].ap().rearrange("(h p) n -> p h n", p=P)

    # PE warm-up: keep the HAM activity monitor busy during the DMA/stats
    # window so projections and attention run at full clock from the start.
    # Memsets go FIRST so the warm matmuls aren't queued behind the bn_stats
    # chain in DVE program order (PE would idle for the whole DMA phase).
    wu_w = const.tile([P, P], BF16)
    nc.vector.memset(wu_w, 0.0)
    wu_x = const.tile([P, 2 * P], BF16)
    nc.vector.memset(wu_x, 0.0)
    wu_ps = ps.tile([P, QB], F32, name="wu_ps", tag="mm")

    def warm(n):
        for _ in range(n):
            nc.tensor.matmul(
                wu_ps[:, 0:2 * P], lhsT=wu_w, rhs=wu_x, start=True, stop=True
            )

    warm(2)

    # ---- head DMA: x8 (1MB, feeds stats AND all matmuls) goes FIRST in
    # 1024-col per-channel chunks: ch1 leads on both queues so the ACT
    # stats units and the DVE's first units have data earliest. ACT's own
    # HWDGE queue carries NO early DMA (each issue costs ~0.7us of ACT
    # sequencer time and starved the act-table loads in an earlier rev):
    # only the weight pack + one x_sb chunk ride scalar, the rest goes
    # sync + gpsimd (SWDGE).
    # Both-channel column chunks (1024B+ contiguous per partition transfers
    # ~90B/ns/queue; channel-split halves that). A small 512-col first
    # chunk lands ~1.5us earlier so the stats stream starts sooner.
    x8_chunks = [
        (nc.sync, 0, 1), (nc.scalar, 1, 2), (nc.gpsimd, 4, 6),
        (nc.sync, 2, 4), (nc.gpsimd, 6, 8),
    ]
    for q, u0, u1 in x8_chunks:
        sl = (slice(None), slice(None), slice(u0 * QB, u1 * QB))
        q.dma_start(out=x8[sl], in_=x8_src[sl])
    # all four weight matrices (bf16) in one DMA on the scalar queue (its
    # only early issue; needed at ~17us for the w_s scaling)
    wstg = stage.tile([P, 4, 2, C], BF16, name="wstg", tag="wstg")
    nc.scalar.dma_start(
        out=wstg, in_=d["wpack"].ap().rearrange("(w h p) co -> p w h co", p=P, h=2)
    )
    m1_sb = const.tile([P, G // 2], F32)
    nc.sync.dma_start(out=m1_sb, in_=d["m1"][:, :])
    m2_sb = const.tile([G // 2, P], F32)
    nc.sync.dma_start(out=m2_sb, in_=d["m2"][:, :])
    # packed biases/affine, pre-permuted on host to [128, 12] with column
    # j = 2*s + co for s in (bq,bk,bv,bo,gamma,beta)
    bcols = const.tile([P, 12], F32)
    nc.sync.dma_start(out=bcols, in_=d["bpack"][:, :])
    xsbq = [nc.sync, nc.sync, nc.gpsimd, nc.gpsimd]
    for c in range(4):
        sl = (slice(None), slice(None), slice(c * 2 * QB, (c + 1) * 2 * QB))
        xsbq[c].dma_start(out=x_sb[sl], in_=x_src[sl])

    # GroupNorm stats from the fp8 x (self-consistent: the projections
    # consume the same fp8 values; adds ~0.1% to the error budget). Split
    # across DVE (bn_stats: ch0 + ch1 units 0,1,2,3,7) and ACT (Square +
    # Identity passes with free-axis accumulate: ch1 units 4,5,6). Every
    # ACT table set contains identity+copy+square, so the sqrt set
    # (preloaded by the dummy sqrt below) serves the stats AND the dance;
    # the only other table load is the exp set, pinned into the projection
    # phase by the dummy exp's fake bo_eff dependency.
    eps16 = const.tile([G // 2, 1], F32)
    nc.vector.memset(eps16, EPS)
    one11 = const.tile([1, 1], F32)
    nc.vector.memset(one11, 1.0)
    warm11 = small.tile([1, 1], F32)
    nc.scalar.activation(warm11, one11, Act.Sqrt, scale=1.0)

    bn_st0 = small.tile([P, NNB, 6], F32, name="bn_st0")
    bn_st1b = small.tile([P, 5, 6], F32, name="bn_st1b")
    ACT_UNITS = (4, 5, 6)
    sq5 = small.tile([P, len(ACT_UNITS)], F32, name="sq5")
    sm5 = small.tile([P, len(ACT_UNITS)], F32, name="sm5")
    scr_bf = stage.tile([P, QB], BF16, name="scr_bf", tag="scr")
    for i, u in enumerate(ACT_UNITS):
        cols = slice(u * QB, (u + 1) * QB)
        # scale folded into the activation: Square(x/64) sums x^2/4096,
        # Identity(x/4096) sums x/4096 — saves two DVE ops in the dance.
        nc.scalar.activation(
            scr_bf, x8[:, 1, cols], Act.Square, scale=1.0 / 64.0,
            accum_out=sq5[:, i:i + 1]
        )
        nc.scalar.activation(
            scr_bf, x8[:, 1, cols], Act.Identity, scale=1.0 / 4096.0,
            accum_out=sm5[:, i:i + 1]
        )
    # DVE bn_stats in data-arrival order: small first chunk (u0), u1,
    # then gpsimd chunk (ch0 u4,u5), sync chunk (u2,u3), gpsimd tail.
    dve_units = [(1, 0, 0), (0, 0, None), (1, 1, 1), (0, 1, None),
                 (0, 4, None), (0, 5, None),
                 (1, 2, 2), (1, 3, 3), (0, 2, None), (0, 3, None),
                 (1, 7, 4), (0, 6, None), (0, 7, None)]
    for i, (ch, u, b1) in enumerate(dve_units):
        dst = bn_st0[:, u, :] if ch == 0 else bn_st1b[:, b1, :]
        nc.vector.bn_stats(out=dst, in_=x8[:, ch, u * QB:(u + 1) * QB])
        if i % 2 == 1:
            warm(6)

    # ---- constants ----
    # padded to 16B so the DoubleRow k-tile stride meets walrus' step%16==0.
    # Value 1.0 balances the at_sb 1/8 down-scale and the x8 wo lift:
    # po*den_r = (8/8)*wo@at_un / den = wo@at_un/den.
    ones8_pad = const.tile([P, 2, 16], FP8)
    nc.vector.memset(ones8_pad, 1.0)
    ones8 = ones8_pad[:, :, 0:1]
    esh_col = const.tile([P, 1], F32)
    nc.vector.memset(esh_col, ESHIFT)

    # ---- GroupNorm dance: per-channel (mean, E[x^2]) -> group reduce via
    # mask matmul -> rstd -> broadcast back via mask matmul -> a/b columns.
    # ch0 comes straight from bn_aggr; ch1 blends the 5 DVE units (2560
    # cols, weight 0.625) with the ACT accumulator sums (1536 cols).
    mvex = small.tile([P, 2, 2], F32, name="mvex")  # (mean, ex2) per ch-half
    mv0 = small.tile([P, 2], F32, name="mv0")
    nc.vector.bn_aggr(out=mv0, in_=bn_st0)
    mv1b = small.tile([P, 2], F32, name="mv1b")
    nc.vector.bn_aggr(out=mv1b, in_=bn_st1b)
    scol = small.tile([P, 2], F32, name="scol")
    nc.vector.tensor_reduce(
        out=scol[:, 0:1], in_=sm5, axis=Axis.X, op=Alu.add
    )
    nc.vector.tensor_reduce(
        out=scol[:, 1:2], in_=sq5, axis=Axis.X, op=Alu.add
    )
    tcol = small.tile([P, 6], F32, name="tcol")
    # ch0: mean; ex2 = mean*mean + var in one scalar-AP STT
    nc.vector.tensor_copy(out=mvex[:, 0, 0:1], in_=mv0[:, 0:1])
    nc.vector.scalar_tensor_tensor(
        out=mvex[:, 0, 1:2], in0=mv0[:, 0:1], scalar=mv0[:, 0:1],
        in1=mv0[:, 1:2], op0=Alu.mult, op1=Alu.add,
    )
    # ch1: mean = FB*mean_b + sum_id/4096 (scol already carries /4096)
    FB = 2560.0 / 4096.0
    nc.vector.scalar_tensor_tensor(
        out=mvex[:, 1, 0:1], in0=mv1b[:, 0:1], scalar=FB, in1=scol[:, 0:1],
        op0=Alu.mult, op1=Alu.add,
    )
    # ch1: ex2 = FB*(mean_b^2 + var_b) + sum_sq/4096
    nc.vector.scalar_tensor_tensor(
        out=tcol[:, 3:4], in0=mv1b[:, 0:1], scalar=mv1b[:, 0:1],
        in1=mv1b[:, 1:2], op0=Alu.mult, op1=Alu.add,
    )
    nc.vector.scalar_tensor_tensor(
        out=mvex[:, 1, 1:2], in0=tcol[:, 3:4], scalar=FB, in1=scol[:, 1:2],
        op0=Alu.mult, op1=Alu.add,
    )
    # group sums over partitions: [16 groups, (mean0, ex20, mean1, ex21)]
    gsum = ps_o.tile([G // 2, 4], F32, name="gsum", tag="po")
    nc.tensor.matmul(
        gsum, lhsT=m1_sb, rhs=mvex.rearrange("p c k -> p (c k)"),
        start=True, stop=True,
    )
    warm(8)
    vals = small.tile([G // 2, 4], F32, name="vals")  # (rstd0, rstd1, m0, m1)
    gtmp = small.tile([G // 2, 4], F32, name="gtmp")
    gview = gsum.rearrange("g (c k) -> g c k", k=2)
    nc.vector.tensor_scalar_mul(
        vals.rearrange("g (c k) -> g c k", k=2)[:, 1, :], gview[:, :, 0],
        1.0 / CPG,
    )
    nc.vector.tensor_scalar_mul(gtmp[:, 0:2], gview[:, :, 1], 1.0 / CPG)
    nc.vector.tensor_mul(gtmp[:, 2:4], vals[:, 2:4], vals[:, 2:4])
    nc.vector.tensor_sub(gtmp[:, 0:2], gtmp[:, 0:2], gtmp[:, 2:4])
    # rstd = 1/sqrt(var + eps) (sqrt table preloaded; recip on DVE)
    nc.scalar.activation(gtmp[:, 2:4], gtmp[:, 0:2], Act.Sqrt, bias=eps16)
    nc.vector.reciprocal(vals[:, 0:2], gtmp[:, 2:4])
    # broadcast back to channels: [128, (rstd0, rstd1, m0, m1)]
    bc = ps_acc.tile([P, 4], F32, name="bc", tag="acc")
    nc.tensor.matmul(bc, lhsT=m2_sb, rhs=vals, start=True, stop=True)
    warm(8)
    # a8 = (8*gamma)*rstd directly (host pre-multiplies gamma by 8);
    # b = beta - (mean*a8)/8. a8 comes first: it is the only gate on the
    # wkt scaling and thus on the first projection matmul.
    cols8 = small.tile([P, 8], F32, name="cols8")
    b_cols = [cols8[:, 2:3], cols8[:, 3:4]]
    a8_cols = [cols8[:, 4:5], cols8[:, 5:6]]
    for ch in range(2):
        nc.vector.tensor_mul(a8_cols[ch], bcols[:, 8 + ch:9 + ch],
                             bc[:, ch:ch + 1])
    for ch in range(2):
        nc.vector.tensor_mul(cols8[:, 6 + ch:7 + ch], bc[:, 2 + ch:3 + ch],
                             a8_cols[ch])
        nc.vector.tensor_scalar(
            out=b_cols[ch], in0=cols8[:, 6 + ch:7 + ch],
            scalar1=-1.0 / WS, scalar2=bcols[:, 10 + ch:11 + ch],
            op0=Alu.mult, op1=Alu.add,
        )
    warm(8)
    # a zero column that data-depends on the dance: used to PIN late ACT
    # ops (wo8) behind the stats/dance — without a dep the scheduler
    # hoists them into the ACT stats stream where their wstg wait stalls it.
    zcol = small.tile([P, 1], F32, name="zcol")
    nc.vector.tensor_scalar_mul(zcol, a8_cols[0], 0.0)
    # HAM filler through the dance: the dep-free warm matmuls all get
    # scheduled during the stats phase, so the PE goes quiet for the
    # ~17-22us dance window and HAM halves the clock right as the
    # projections start. These reads of the (just-landed) weight pack are
    # naturally pinned to that window.
    for i in range(28):
        nc.tensor.matmul(
            wu_ps[:, 0:C], lhsT=wu_w, rhs=wstg[:, i % 4, i % 2, :],
            start=True, stop=True,
        )

    # scale wq/wk/wv rows by 8*a (per input channel) straight from the f32
    # staging into fp8 tiles. The x8 lift keeps the fp8 entries
    # (~N(0, a/16)) out of e4m3 subnormal range; the PSUM->SBUF casts
    # divide it back out.
    w_s = {}
    for wi, wname in ((0, "wqt"), (1, "wkt"), (2, "wvt")):
        w_s[wname] = const.tile([P, 2, C], FP8, name=f"{wname}_s")
    # wk first: it gates the longest projection stream (all on DVE — the
    # Pool engine's ucode tensor ops force library swaps that stall the
    # whole core).
    for wname, wi in (("wkt", 1), ("wqt", 0), ("wvt", 2)):
        for ci in range(2):
            nc.vector.tensor_scalar_mul(
                w_s[wname][:, ci, :], wstg[:, wi, ci, :], a8_cols[ci]
            )
    wo8 = const.tile([P, 2, C], FP8)

    # ---- projections (all DoubleRow fp8, contraction over full C=256) ----
    # K^T [C, N] fp8: psum[co,nb] = sum_ci wkt8[ci,co].T @ x8[ci, nb] (x8)
    # and q,k = psum/8 + bias. Casts alternate ACT / DVE (ACT is idle until
    # the attention loop's exp stream starts). The first K matmul pair is
    # issued the moment wkt is scaled; the tiny bias matvecs then ride the
    # PE behind it so their serial chain overlaps the projection stream.
    k_sb = const.tile([P, 2, N], FP8)
    q_sb = const.tile([P, 2, NQ], FP8)
    IWS = 1.0 / WS
    proj_ps = [(ps, "mm"), (ps_acc, "acc"), (ps, "mm"), (ps_o, "po"),
               (ps_acc, "acc")]
    _pi = [0]

    def proj_tile():
        pool, tag = proj_ps[_pi[0] % len(proj_ps)]
        _pi[0] += 1
        return pool.tile([P, QB], F32, name="pp", tag=tag)

    _ci = [0]

    def proj_cast(out, in_, bias):
        # only ACT/DVE can read PSUM; the 5-bank psum ring above keeps the
        # MM stream ~2.5 tiles ahead so the 2-engine cast pace (~325ns/tile)
        # doesn't stall the PE.
        i = _ci[0] % 2
        _ci[0] += 1
        if i == 0:
            nc.scalar.activation(out, in_, Act.Identity, bias=bias, scale=IWS)
        else:
            nc.vector.tensor_scalar(
                out=out, in0=in_, scalar1=IWS, scalar2=bias, op0=Alu.mult,
                op1=Alu.add,
            )

    # first K pair (block 0) — only these two proj psums may be in flight
    # before the matvec chain (deeper would deadlock through be_*'s ring
    # reuse of the same PSUM tags).
    pk0 = [proj_tile() for _ in range(2)]
    for co in range(2):
        nc.tensor.matmul(
            pk0[co], lhsT=w_s["wkt"][:, :, co * P:(co + 1) * P],
            rhs=x8[:, :, 0:QB], start=True, stop=True, perf_mode=DR,
        )

    # projection bias columns: be = W b + bias. All 12 first-level matvecs
    # (wq/wk/wv x co x ci) land in disjoint columns of ONE psum tile, so
    # the PE runs them back-to-back with no ring WARs or ACT round-trips
    # (the old 3-pool ring chain left the PE sparse enough that HAM halved
    # the clock); one strided DVE add applies all six biases at once.
    b_bf = small.tile([P, 2], BF16, name="b_bf")
    for ch in range(2):
        nc.vector.tensor_copy(out=b_bf[:, ch:ch + 1], in_=b_cols[ch])
    mv8 = ps.tile([P, 8], F32, name="mv8", tag="mm")
    for wi in range(3):
        for co in range(2):
            j = 2 * wi + co
            for ci in range(2):
                nc.tensor.matmul(
                    mv8[:, j:j + 1],
                    lhsT=wstg[:, wi, ci, co * P:(co + 1) * P],
                    rhs=b_bf[:, ci:ci + 1], start=(ci == 0), stop=(ci == 1),
                    skip_group_check=True,
                )
    be6 = small.tile([P, 6], F32, name="be6")
    nc.vector.tensor_add(be6, mv8[:, 0:6], bcols[:, 0:6])
    be_q = [be6[:, 0:1], be6[:, 1:2]]
    be_k = [be6[:, 2:3], be6[:, 3:4]]
    v_bf = small.tile([P, 2], BF16, name="v_bf")
    nc.vector.tensor_copy(out=v_bf, in_=be6[:, 4:6])
    # second level: bo_eff = wo @ vbv + bo (4 more dense matmuls)
    for co in range(2):
        for ci in range(2):
            nc.tensor.matmul(
                mv8[:, 6 + co:7 + co],
                lhsT=wstg[:, 3, ci, co * P:(co + 1) * P],
                rhs=v_bf[:, ci:ci + 1], start=(ci == 0), stop=(ci == 1),
                skip_group_check=True,
            )
    bo2 = small.tile([P, 2], F32, name="bo2")
    nc.vector.tensor_add(bo2, mv8[:, 6:8], bcols[:, 6:8])
    bo_eff = [bo2[:, 0:1], bo2[:, 1:2]]
    # dummy exp: pulls the exp table load into the projection phase. The
    # scale AP is a fake data dependency on bo_eff — without it the tile
    # scheduler hoists this exp into the stats phase and the sqrt/exp
    # table sets ping-pong (4 extra 1.5us table loads).
    nc.scalar.activation(warm11, one11, Act.Exp, scale=bo2[0:1, 0:1])
    for co in range(2):
        proj_cast(k_sb[:, co, 0:QB], pk0[co], be_k[co])
    # wo8 = 8*wo in fp8 (no GroupNorm folding on the out-proj); the zcol
    # bias pins these behind the dance (first consumer is qb0's epilogue,
    # ~20us later).
    for ch in range(2):
        nc.scalar.activation(wo8[:, ch, :], wstg[:, 3, ch, :], Act.Identity,
                             bias=zcol, scale=WS)

    for nb in range(NNB):
        for co in range(2):
            if nb < NQB:
                pq = proj_tile()
                nc.tensor.matmul(
                    pq, lhsT=w_s["wqt"][:, :, co * P:(co + 1) * P],
                    rhs=x8[:, :, nb * QB:(nb + 1) * QB],
                    start=True, stop=True, perf_mode=DR,
                )
                proj_cast(q_sb[:, co, nb * QB:(nb + 1) * QB], pq, be_q[co])
            if nb >= 1:
                pk = proj_tile()
                nc.tensor.matmul(
                    pk, lhsT=w_s["wkt"][:, :, co * P:(co + 1) * P],
                    rhs=x8[:, :, nb * QB:(nb + 1) * QB],
                    start=True, stop=True, perf_mode=DR,
                )
                proj_cast(k_sb[:, co, nb * QB:(nb + 1) * QB], pk, be_k[co])

    # V [N, C] fp8 (bias folded into bo_eff): psum[nt] = x8_tile.T @ wvt8
    v_sb = const.tile([P, NKT, C], FP8)
    v_flat = v_sb.rearrange("p k c -> p (k c)")
    zero_col = const.tile([P, 1], F32)
    nc.vector.memset(zero_col, 0.0)
    for nt in range(0, NKT, 2):
        pv = proj_tile()
        for n2 in range(2):
            nc.tensor.matmul(
                pv[:, n2 * C:(n2 + 1) * C],
                lhsT=x8[:, :, (nt + n2) * P:(nt + n2 + 1) * P],
                rhs=w_s["wvt"][:, :, :],
                start=True, stop=True, perf_mode=DR,
            )
        proj_cast(v_flat[:, nt * C:(nt + 2) * C], pv, zero_col)

    # ---- attention, per query block; DoubleRow over key-tile pairs with a
    # batched exp (one ACT call per pair reading a 2-bank PSUM tile) and the
    # denominator as a DoubleRow ones-matmul on the PE. The softmax division
    # is commuted through the out-projection: out = (wo8 @ PV) * (1/(8*den))
    # + bo_eff + x, deferred one qb so nothing waits on the reciprocal.
    def epilogue_a(qb, dps, aps, at_sb, p_sb_dbg=None):
        # casts first: they release the PV accumulator banks immediately.
        # 1/8 keeps the heavy-tailed PV numerator inside fp8's +-240.
        nc.vector.tensor_scalar_mul(at_sb[:, 0, :], aps[0], 1.0 / 8.0)
        nc.vector.tensor_scalar_mul(at_sb[:, 1, :], aps[1], 1.0 / 8.0)
        den_r = work.tile([1, QB], F32, name="den_r", tag="den_r")
        nc.vector.reciprocal_approx_fast(out=den_r, in_=dps)
        den_b = work.tile([P, QB], F32, name="den_b", tag="den_b", bufs=2)
        nc.gpsimd.partition_broadcast(den_b, den_r)
        if DEBUG:
            nc.sync.dma_start(
                out=d["dbg_denr"][:, qb * QB:(qb + 1) * QB], in_=den_r
            )
            nc.sync.dma_start(
                out=d["dbg_at"][:, qb * 2 * QB:(qb + 1) * 2 * QB],
                in_=at_sb.rearrange("p h n -> p (h n)"),
            )
            if qb == 0:
                nc.sync.dma_start(
                    out=d["dbg_p"][:, :], in_=p_sb_dbg.rearrange("p k n -> p (k n)")
                )
        return den_b

    def epilogue_co(qb, at_sb, den_b, co, po_pool=None, po_tag="po",
                    dma_split=False):
        po_pool = po_pool or ps_o
        po = po_pool.tile([P, QB], F32, name="po", tag=po_tag)
        nc.tensor.matmul(
            po, lhsT=wo8[:, :, co * P:(co + 1) * P], rhs=at_sb,
            start=True, stop=True, perf_mode=DR,
        )
        t1 = work.tile([P, QB], F32, name="t1", tag="t1")
        nc.vector.tensor_mul(t1, po, den_b)
        res = work.tile([P, QB], F32, name="res", tag="res", bufs=4)
        nc.vector.scalar_tensor_tensor(
            out=res, in0=t1, scalar=bo_eff[co],
            in1=x_sb[:, co, qb * QB:(qb + 1) * QB], op0=Alu.add, op1=Alu.add,
        )
        if dma_split:
            # tail-exposed final block: halve the transfer across both
            # HWDGE queues so the last bytes land ~1.5us earlier.
            HB = QB // 2
            for h, qeng in ((0, nc.sync), (1, nc.scalar)):
                qeng.dma_start(
                    out=out_d[co * P:(co + 1) * P,
                              qb * QB + h * HB:qb * QB + (h + 1) * HB],
                    in_=res[:, h * HB:(h + 1) * HB],
                )
        else:
            nc.sync.dma_start(
                out=out_d[co * P:(co + 1) * P, qb * QB:(qb + 1) * QB], in_=res
            )

    if DEBUG:
        nc.sync.dma_start(out=d["dbg_k"][:, :], in_=k_sb.rearrange("p h n -> p (h n)"))
        nc.sync.dma_start(out=d["dbg_q"][:, :], in_=q_sb.rearrange("p h n -> p (h n)"))
        nc.sync.dma_start(out=d["dbg_v"][:, :], in_=v_flat)

    pending = None
    pend_den = None
    for qb in range(NQB):
        p_sb = pblk.tile([P, NKT, QB], FP8, name="p_sb")
        dps = ps_d.tile([1, QB], F32, name="dps")
        aps = [
            ps_acc.tile([P, QB], F32, name="aps", tag="acc") for _ in range(2)
        ]
        at_sb = work.tile([P, 2, QB], FP8, name="at_sb", tag="at_sb", bufs=2)
        for j in range(NKP + 2):
            if j == 1 and pending is not None:
                pend_den = epilogue_a(*pending)
            if j == 2 and pending is not None:
                epilogue_co(pending[0], pending[3], pend_den, 0)
            if j == 3 and pending is not None:
                epilogue_co(pending[0], pending[3], pend_den, 1)
                pending = None
            if j < NKP:
                sps2 = ps.tile([P, 2 * QB], F32, name="sps2", tag="mm")
                for h2 in range(2):
                    kt = 2 * j + h2
                    nc.tensor.matmul(
                        sps2[:, h2 * QB:(h2 + 1) * QB],
                        lhsT=k_sb[:, :, kt * P:(kt + 1) * P],
                        rhs=q_sb[:, :, qb * QB:(qb + 1) * QB],
                        start=True, stop=True, perf_mode=DR,
                    )
                nc.scalar.activation(
                    p_sb[:, 2 * j:2 * j + 2, :], sps2, Act.Exp,
                    bias=esh_col, scale=SCALE,
                )
            if j >= 2:
                pj = j - 2
                pair = p_sb[:, 2 * pj:2 * pj + 2, :]
                for ch in range(2):
                    nc.tensor.matmul(
                        aps[ch],
                        lhsT=v_sb[:, 2 * pj:2 * pj + 2, ch * P:(ch + 1) * P],
                        rhs=pair,
                        start=(pj == 0), stop=(pj == NKP - 1),
                        perf_mode=DR, skip_group_check=True,
                    )
                nc.tensor.matmul(
                    dps, lhsT=ones8, rhs=pair,
                    start=(pj == 0), stop=(pj == NKP - 1),
                    perf_mode=DR, skip_group_check=True,
                )
        pending = (qb, dps, aps, at_sb, p_sb)
    pend_den = epilogue_a(*pending)
    epilogue_co(pending[0], pending[3], pend_den, 0, po_pool=ps, po_tag="mm",
                dma_split=True)
    epilogue_co(pending[0], pending[3], pend_den, 1, po_pool=ps, po_tag="mm",
                dma_split=True)

    for pool in (ps_o, ps_d, ps_acc, ps, work, pblk, small, stage, const):
        pool.release()


def build_program():
    global _NC
    if _NC is not None:
        return _NC
    nc = bacc.Bacc("TRN2", target_bir_lowering=False, debug=False,
                   num_devices=NCORES)
    d = {
        "x": nc.dram_tensor("x", [C, N], BF16, kind="ExternalInput"),
        "x8": nc.dram_tensor("x8", [C, N], FP8, kind="ExternalInput"),
        "wpack": nc.dram_tensor("wpack", [4 * C, C], BF16, kind="ExternalInput"),
        "bpack": nc.dram_tensor("bpack", [P, 12], F32, kind="ExternalInput"),
        "m1": nc.dram_tensor("m1", [P, G // 2], F32, kind="ExternalInput"),
        "m2": nc.dram_tensor("m2", [G // 2, P], F32, kind="ExternalInput"),
        "out": nc.dram_tensor("out", [C, NQ], F32, kind="ExternalOutput"),
    }
    if DEBUG:
        d.update({
            "dbg_k": nc.dram_tensor("dbg_k", [P, 2 * N], FP8, kind="ExternalOutput"),
            "dbg_q": nc.dram_tensor("dbg_q", [P, 2 * NQ], FP8, kind="ExternalOutput"),
            "dbg_v": nc.dram_tensor("dbg_v", [P, NKT * C], FP8, kind="ExternalOutput"),
            "dbg_p": nc.dram_tensor("dbg_p", [P, NKT * QB], FP8, kind="ExternalOutput"),
            "dbg_denr": nc.dram_tensor("dbg_denr", [1, NQ], F32, kind="ExternalOutput"),
            "dbg_at": nc.dram_tensor("dbg_at", [P, NQB * 2 * QB], FP8, kind="ExternalOutput"),
        })
    with tile.TileContext(nc) as tc:
        _body(tc, d)
    nc.compile()
    _NC = nc
    return nc


def make_in_maps(x, gamma, beta, wq, bq, wk, bk, wv, bv, wo, bo):
    import ml_dtypes

    f32c = lambda a: np.ascontiguousarray(np.asarray(a, dtype=np.float32))
    x = f32c(x)
    wpack = np.concatenate(
        [f32c(np.asarray(w, np.float32).T) for w in (wq, wk, wv, wo)], axis=0
    )
    bp6 = np.stack(
        [f32c(v).reshape(C) for v in (bq, bk, bv, bo, gamma, beta)], axis=1
    )  # [C, 6]
    bpack = np.empty((P, 12), np.float32)
    for s in range(6):
        for co in range(2):
            bpack[:, 2 * s + co] = bp6[co * P:(co + 1) * P, s]
    bpack[:, 8:10] *= 8.0  # gamma pre-scaled so a8 is a single runtime mul
    m1 = np.zeros((P, G // 2), np.float32)
    for g in range(G // 2):
        m1[8 * g:8 * g + 8, g] = 1.0
    base = {
        "wpack": np.ascontiguousarray(wpack.astype(ml_dtypes.bfloat16)),
        "bpack": np.ascontiguousarray(bpack),
        "m1": m1,
        "m2": np.ascontiguousarray(m1.T),
    }
    import ml_dtypes

    in_maps = []
    for core in range(NCORES):
        b, h = divmod(core, 2)
        xb = x[b].reshape(C, N)
        if h:
            xb = np.concatenate([xb[:, NQ:], xb[:, :NQ]], axis=1)
        in_maps.append({
            **base,
            "x": np.ascontiguousarray(xb.astype(ml_dtypes.bfloat16)),
            "x8": np.ascontiguousarray(xb.astype(ml_dtypes.float8_e4m3)),
        })
    return in_maps


def kernel(x, gamma, beta, wq, bq, wk, bk, wv, bv, wo, bo):
    global LAST_RESULTS
    from concourse.bass_utils import run_bass_kernel_spmd

    nc = build_program()
    in_maps = make_in_maps(x, gamma, beta, wq, bq, wk, bk, wv, bv, wo, bo)
    res = run_bass_kernel_spmd(nc, in_maps, core_ids=list(range(NCORES)))
    LAST_RESULTS = res
    out = np.empty((B, C, N), np.float32)
    for core in range(NCORES):
        b, h = divmod(core, 2)
        out[b][:, h * NQ:(h + 1) * NQ] = res.results[core]["out"]
    return out.reshape(B, C, H, W)

